# revision 1
# baseline (speedup 1.0000x reference)
"""Multi-head attention (multiquery K/V) Bass kernel for 8 trn2 NeuronCores.

Sharding: 8 cores = 2 batches x 4 query-row quarters. Each core computes the
full multiquery K/V projections for its batch (cheap, dk=64) and runs
attention + output projection for its 512 query rows over all 16 heads.
Output is a pure concatenation across cores -- no collectives.

Device data flow (everything "transposed" so matmuls contract on partitions):
  xT [D, S] (host-transposed; this core's query rows rolled to columns 0:512)
  QT [dout, s]  = wqT.T-contract (x @ w_q.T transposed)
  K2T [128, t]  = K.T stacked twice (rows 0:64 and 64:128) so even/odd heads
                  can use row-group-aligned operands
  Vp [t, 65]    = V with a ones column (column 64 accumulates the softmax
                  denominator during the attn@V matmul)
  scores_T[t,s] per head -> exp on ACT (scale folded in) -> attn@V transposed
  normalize via reciprocal + rank-1 matmul partition-broadcast
  y = out @ w_out.T via Kc=64 accumulation over heads
"""

import sys

import numpy as np

if "/opt/trn_rl_repo" not in sys.path:
    sys.path.insert(0, "/opt/trn_rl_repo")

B, S, D = 2, 2048, 1024
H, DK = 16, 64
P = 128
NCORES, GPB = 8, 4
SPB = S // GPB  # 512 query rows per core
KC = D // P  # 8 contraction subtiles over d_model
NT = S // P  # 16 key/t blocks
NSB = SPB // P  # 4 s blocks
NDF = D // 512  # 2 output column halves


USE_F32R = True


def build_bass(scale: float):
    import concourse.bacc as bacc
    import concourse.mybir as mybir
    import concourse.tile as tile
    from concourse.bass import ts

    fp32 = mybir.dt.float32
    # matmul-operand dtype: float32r streams at 1 cycle/row (vs 4 for
    # float32) and is bit-identical fp32 in memory
    mdt = mybir.dt.float32r if USE_F32R else fp32
    Act = mybir.ActivationFunctionType

    nc = bacc.Bacc(None, target_bir_lowering=False)
    xT = nc.dram_tensor("xT", [D, S], mdt, kind="ExternalInput")
    cst = nc.dram_tensor("cst", [P, P], mdt, kind="ExternalInput")
    wqT = nc.dram_tensor("wqT", [D, D], mdt, kind="ExternalInput")
    wkkT = nc.dram_tensor("wkkT", [D, P], mdt, kind="ExternalInput")
    wvT = nc.dram_tensor("wvT", [D, DK], mdt, kind="ExternalInput")
    wo64 = nc.dram_tensor("wo64", [DK, H, D], mdt, kind="ExternalInput")
    y = nc.dram_tensor("y", [SPB, D], fp32, kind="ExternalOutput")

    xT3 = xT.rearrange("(po pi) s -> pi po s", pi=P)
    wq3 = wqT.rearrange("(po pi) d -> pi po d", pi=P)
    wkk3 = wkkT.rearrange("(po pi) d -> pi po d", pi=P)
    wv3 = wvT.rearrange("(po pi) d -> pi po d", pi=P)

    with tile.TileContext(nc) as tc:
        with tc.tile_pool(name="persist", bufs=1) as pers:
            K2T = pers.tile([P, S], mdt, name="K2T")
            Vp = pers.tile([P, NT, DK + 1], mdt, name="Vp")
            QT = pers.tile([P, KC, SPB], mdt, name="QT")
            oT = pers.tile([DK, H, SPB], mdt, name="oT")
            cst_sb = pers.tile([P, P], mdt, name="cst")
            # cst cols 0:64 = identity(64) (rows 0:64), cols 64:128 = ones
            nc.gpsimd.dma_start(cst_sb[:], cst[:])
            nc.gpsimd.dma_start(Vp[:, :, DK], cst[:, DK:DK + NT])
            ident = cst_sb[0:DK, 0:DK]
            ones = cst_sb[:, DK:DK + DK]

            # ---------------- Phase A: projections ----------------
            with (
                tc.tile_pool(name="pa", bufs=1) as pa,
                tc.tile_pool(name="paps", bufs=1, space="PSUM") as paps,
            ):
                wkk_sb = pa.tile([P, KC, P], mdt, name="wkk")
                wv_sb = pa.tile([P, KC, DK], mdt, name="wv")
                nc.gpsimd.dma_start(wkk_sb[:], wkk3[:])
                nc.gpsimd.dma_start(wv_sb[:], wv3[:])
                wq_sb = pa.tile([P, KC, D], mdt, name="wq")
                # split the 4MB w_q load so Q matmuls can start early
                for half in range(2):
                    nc.gpsimd.dma_start(
                        wq_sb[:, :, ts(half, 512)], wq3[:, :, ts(half, 512)]
                    )

                for c in range(4):
                    xc = pa.tile([P, KC, 512], mdt, name="xc", tag="xc", bufs=2)
                    nc.gpsimd.dma_start(xc[:], xT3[:, :, ts(c, 512)])

                    k2ps = paps.tile([P, 512], fp32, name="k2ps", tag="k2ps", bufs=2)
                    for k in range(KC):
                        nc.tensor.matmul(
                            k2ps[:], wkk_sb[:, k, :], xc[:, k, :],
                            start=(k == 0), stop=(k == KC - 1),
                        )
                    nc.vector.tensor_copy(K2T[:, ts(c, 512)], k2ps[:])

                    # V.T then PE-transpose into V' natural [t, dv]
                    vps = paps.tile([DK, 512], fp32, name="vps", tag="vps", bufs=2)
                    for k in range(KC):
                        nc.tensor.matmul(
                            vps[:], wv_sb[:, k, :], xc[:, k, :],
                            start=(k == 0), stop=(k == KC - 1),
                        )
                    vsb = pa.tile([DK, 512], mdt, name="vsb", tag="vsb", bufs=2)
                    nc.vector.tensor_copy(vsb[:], vps[:])
                    for q in range(4):
                        trps = paps.tile([P, DK], mdt, name="trps", tag="trps", bufs=2)
                        nc.tensor.transpose(trps[:], vsb[:, ts(q, P)], ident)
                        nc.vector.tensor_copy(Vp[:, 4 * c + q, 0:DK], trps[:])

                    if c == 0:
                        for m in range(KC):
                            qps = paps.tile([P, 512], fp32, name="qps", tag="qps", bufs=2)
                            for k in range(KC):
                                nc.tensor.matmul(
                                    qps[:], wq_sb[:, k, ts(m, P)], xc[:, k, :],
                                    start=(k == 0), stop=(k == KC - 1),
                                )
                            nc.vector.tensor_copy(QT[:, m, :], qps[:])

            # ---------------- Phase B: attention ----------------
            with tc.tile_pool(name="pb", bufs=1) as pb:
                wo_sb = pb.tile([DK, H, D], mdt, name="wo")
                nc.gpsimd.dma_start(wo_sb[:], wo64[:])

                with tc.tile_pool(name="pbps", bufs=1, space="PSUM") as pbps:
                    for p4 in range(4):
                        # outps[g][l]: attn@V accumulator for head pair g and
                        # s-half l: [65, 2(head), 256(s)] = one PSUM bank; one
                        # matmul covers both heads of the pair (rhs packs the
                        # two heads' exp side by side at N=512)
                        outps = [
                            [
                                pbps.tile([DK + 1, 2, 256], fp32,
                                          name=f"outps{g}{l}",
                                          tag=f"outps{g}{l}", bufs=1)
                                for l in range(2)
                            ]
                            for g in range(2)
                        ]
                        # software pipeline: scores(tb)+exp(tb) are emitted
                        # before attnV(tb-1), so the PE never sits waiting
                        # for the exp of the block it just produced
                        exs = {}

                        def emit_attnv(tb):
                            for g in range(2):
                                for l in range(2):
                                    nc.tensor.matmul(
                                        outps[g][l][:], Vp[:, tb, :],
                                        exs[(tb, g)][:, :, ts(l, 256)],
                                        start=(tb == 0), stop=(tb == NT - 1),
                                    )

                        for tb in range(NT):
                            for g in range(2):
                                sc = pbps.tile([P, 2, 512], fp32, name=f"sc{g}",
                                               tag=f"sc{g}", bufs=1)
                                for j in range(2):
                                    h = 4 * p4 + 2 * g + j
                                    r = (h % 2) * DK
                                    nc.tensor.matmul(
                                        sc[:, j, :],
                                        K2T[r:r + DK, ts(tb, P)],
                                        QT[r:r + DK, h // 2, :],
                                        start=True, stop=True,
                                    )
                                ex = pb.tile([P, 2, 512], mdt, name=f"ex{g}",
                                             tag=f"ex{g}", bufs=6)
                                nc.scalar.activation(ex[:], sc[:], Act.Exp, scale=scale)
                                exs[(tb, g)] = ex
                            if tb > 0:
                                emit_attnv(tb - 1)
                        emit_attnv(NT - 1)
                        # bc slots rotate through psum tags freed in order
                        # (sc1 after the last exp, then each outps slot as its
                        # pair's normalize completes) so the NEXT pass's
                        # scores on sc0 are never blocked by normalization
                        bc_tags = ["sc1", "sc0", "outps00", "outps01"]
                        for hh in range(4):
                            g, j = hh // 2, hh % 2
                            h = 4 * p4 + 2 * g + j
                            rec = pb.tile([DK + 1, 512], mdt, name="rec",
                                          tag="rec", bufs=4)
                            with nc.allow_low_precision(
                                reason="float32r is 4-byte fp32 storage"
                            ):
                                for l in range(2):
                                    nc.vector.reciprocal(
                                        rec[DK:DK + 1, ts(l, 256)],
                                        outps[g][l][DK:DK + 1, j, :],
                                    )
                            bc = pbps.tile([DK, 512], fp32, name="bc",
                                           tag=bc_tags[hh], bufs=1)
                            nc.tensor.matmul(
                                bc[:], ones[DK:DK + 1, 0:DK], rec[DK:DK + 1, :],
                                start=True, stop=True,
                            )
                            bcs = pb.tile([DK, 512], fp32, name="bcs",
                                          tag="bcs", bufs=4)
                            nc.vector.tensor_copy(bcs[:], bc[:])
                            for l in range(2):
                                nc.vector.tensor_mul(
                                    oT[:, h, ts(l, 256)],
                                    outps[g][l][0:DK, j, :],
                                    bcs[:, ts(l, 256)],
                                )

                # ---------------- Phase C: output projection ----------------
                with tc.tile_pool(name="pcps", bufs=1, space="PSUM") as pcps:
                    for sb in range(NSB):
                        for df in range(NDF):
                            yps = pcps.tile([P, 512], fp32, name="yps",
                                            tag="yps", bufs=2)
                            for h in range(H):
                                nc.tensor.matmul(
                                    yps[:],
                                    oT[:, h, ts(sb, P)],
                                    wo_sb[:, h, ts(df, 512)],
                                    start=(h == 0), stop=(h == H - 1),
                                )
                            ysb = pb.tile([P, 512], fp32, name="ysb",
                                          tag="ysb", bufs=3)
                            nc.vector.tensor_copy(ysb[:], yps[:])
                            nc.gpsimd.dma_start(y[ts(sb, P), ts(df, 512)], ysb[:])

    nc.compile()
    return nc


def make_in_maps(x, w_q, w_k, w_v, w_out):
    cst = np.zeros((P, P), dtype=np.float32)
    cst[0:DK, 0:DK] = np.eye(DK, dtype=np.float32)
    cst[:, DK:] = 1.0
    x = np.ascontiguousarray(np.asarray(x, dtype=np.float32))
    w_q = np.asarray(w_q, dtype=np.float32)
    w_k = np.asarray(w_k, dtype=np.float32)
    w_v = np.asarray(w_v, dtype=np.float32)
    w_out = np.asarray(w_out, dtype=np.float32)

    wqT = np.ascontiguousarray(w_q.T)
    wkkT = np.ascontiguousarray(np.concatenate([w_k.T, w_k.T], axis=1))
    wvT = np.ascontiguousarray(w_v.T)
    wo64 = np.ascontiguousarray(w_out.T.reshape(H, DK, D).transpose(1, 0, 2))

    in_maps = []
    for c in range(NCORES):
        b, r = divmod(c, GPB)
        # roll this core's query rows to the front; t-order is irrelevant
        # (attention sums over t), so K/V are unaffected
        xb = np.roll(x[b], -r * SPB, axis=0)
        xTc = np.ascontiguousarray(xb.T)
        in_maps.append(
            {"xT": xTc, "wqT": wqT, "wkkT": wkkT, "wvT": wvT, "wo64": wo64,
             "cst": cst}
        )
    return in_maps


_BUILD_CACHE = {}


def _cached_nc(scale: float):
    key = round(float(scale), 12)
    if key not in _BUILD_CACHE:
        _BUILD_CACHE[key] = build_bass(float(scale))
    return _BUILD_CACHE[key]


def run_on_hw(in_maps, scale, trace=False):
    from concourse.bass_utils import run_bass_kernel_spmd

    nc = _cached_nc(scale)
    return run_bass_kernel_spmd(nc, in_maps, list(range(NCORES)), trace=trace)


def assemble(results):
    out = np.empty((B, S, D), dtype=np.float32)
    for c in range(NCORES):
        b, r = divmod(c, GPB)
        out[b, r * SPB:(r + 1) * SPB] = results[c]["y"]
    return out


def kernel(x, w_q, w_k, w_v, w_out, softmax_scale):
    scale = float(np.asarray(softmax_scale).reshape(-1)[0])
    in_maps = make_in_maps(x, w_q, w_k, w_v, w_out)
    res = run_on_hw(in_maps, scale, trace=False)
    return assemble(res.results)



# revision 27
# speedup vs baseline: 1.2854x; 1.2854x over previous
"""Multi-head attention (multiquery K/V) Bass kernel for 8 trn2 NeuronCores.

Sharding: 8 cores = 2 batches x 4 query-row quarters. Each core computes the
full multiquery K/V projections for its batch (cheap, dk=64) and runs
attention + output projection for its 512 query rows over all 16 heads.
Output is a pure concatenation across cores -- no collectives.

v2 design notes (vs the original phased kernel):
- The Scalar engine's exp over [t=2048, s=512] x 16 heads (~135us at
  1 elem/cycle/lane) is the per-core floor; everything else is arranged to
  hide beneath it.
- All steady-state matmuls use the same (64-row, 128-col) PE tiling mode so
  the array never drains: score pairs run concurrently on row tiles T0/T8;
  attn@V splits its t=128 contraction into two concurrent 64-row halves with
  a skewed issue order so no PSUM bank is touched by both tiles at once.
- 8 passes of one head pair each. Per pass PSUM: sc double buffer (4 banks)
  + attn@V accumulator (2 banks) + 2 generic 1-bank aux slots = 8 banks.
- attn@V keeps the ones-column trick (stationary [V | 1] of width 65) so the
  softmax denominator accumulates in psum row 64 across both halves.
- Normalize: reciprocal_approx_fast of the denominator row into row 0 of a
  zeroed [64, 2, 512] tile, then a ones[64,65]-stationary matmul broadcasts
  it across partitions in the same PE mode. Odd heads multiply into SBUF
  partitions 64:128 (cross-partition DVE write) so the per-pair output oT is
  contraction-ready for the fused output projection.
- Output projection is fused into the pass pipeline: per pass 8 [128s x 512d]
  matmuls + DVE accumulate into y_sb, overlapped with the next pass's exps.
- Projections for c-blocks 1..3 and q-blocks 1..7 are emitted as hooks inside
  early passes, in the same split-contraction mode, filling PE slack.
"""

import sys

import numpy as np

if "/opt/trn_rl_repo" not in sys.path:
    sys.path.insert(0, "/opt/trn_rl_repo")

B, S, D = 2, 2048, 1024
H, DK = 16, 64
H2 = H // 2  # head pairs
P = 128
NCORES, GPB = 8, 4
SPB = S // GPB  # 512 query rows per core
KC = D // P  # 8 contraction subtiles over d_model
NT = S // P  # 16 key/t blocks
NSB = SPB // P  # 4 s blocks


def build_bass(scale: float, debug: bool = False):
    import concourse.bacc as bacc
    import concourse.mybir as mybir
    import concourse.tile as tile
    from concourse.bass import ts

    fp32 = mybir.dt.float32
    mdt = mybir.dt.float32r  # fp32 bits, streams 1 cycle/row on the PE
    Act = mybir.ActivationFunctionType

    nc = bacc.Bacc(None, target_bir_lowering=False)
    xT = nc.dram_tensor("xT", [D, S], mdt, kind="ExternalInput")
    cst = nc.dram_tensor("cst", [P, 256], mdt, kind="ExternalInput")
    wqT = nc.dram_tensor("wqT", [D, D], mdt, kind="ExternalInput")
    wkkT = nc.dram_tensor("wkkT", [D, P], mdt, kind="ExternalInput")
    wvT = nc.dram_tensor("wvT", [D, DK + 1], mdt, kind="ExternalInput")
    wo2 = nc.dram_tensor("wo2", [P, H2, D], mdt, kind="ExternalInput")
    y = nc.dram_tensor("y", [SPB, D], fp32, kind="ExternalOutput")
    if debug:
        dK2T = nc.dram_tensor("dK2T", [P, S], fp32, kind="ExternalOutput")
        dQT = nc.dram_tensor("dQT", [P, KC, SPB], fp32, kind="ExternalOutput")
        dVp = nc.dram_tensor("dVp", [P, NT, DK + 1], fp32,
                             kind="ExternalOutput")
        doT = nc.dram_tensor("doT", [H2, P, SPB], fp32, kind="ExternalOutput")
        dacc = nc.dram_tensor("dacc", [DK + 1, 2, SPB], fp32,
                              kind="ExternalOutput")
        dsc = nc.dram_tensor("dsc", [P, 2, SPB], fp32, kind="ExternalOutput")
        dex = nc.dram_tensor("dex", [P, 2, SPB], fp32, kind="ExternalOutput")
        dbcs = nc.dram_tensor("dbcs", [H2, 2, DK, SPB], fp32,
                              kind="ExternalOutput")

    xT3 = xT.rearrange("(po pi) s -> pi po s", pi=P)
    wq3 = wqT.rearrange("(po pi) d -> pi po d", pi=P)
    wkk3 = wkkT.rearrange("(po pi) d -> pi po d", pi=P)
    wv3 = wvT.rearrange("(po pi) d -> pi po d", pi=P)

    with tile.TileContext(nc) as tc:
        with (
            tc.tile_pool(name="sb", bufs=1) as sb,
            tc.tile_pool(name="ps", bufs=1, space="PSUM") as ps,
        ):
            # ---- persistent SBUF ----
            cst_sb = sb.tile([P, 256], mdt, name="cst")
            K2T = sb.tile([P, S], mdt, name="K2T")
            Vp = sb.tile([P, NT, DK + 1], mdt, name="Vp")
            QT = sb.tile([P, KC, SPB], mdt, name="QT")
            rec64 = sb.tile([DK, 2, SPB], mdt, name="rec64")
            den_sb = sb.tile([1, 2, SPB], fp32, name="den_sb")
            y_sb = sb.tile([P, NSB, D], fp32, name="y_sb")
            wkk_sb = sb.tile([P, KC, P], mdt, name="wkk")
            wv_sb = sb.tile([P, KC, DK + 1], mdt, name="wv")
            wq_sb = sb.tile([P, KC, D], mdt, name="wq")
            wo2_sb = sb.tile([P, H2, D], mdt, name="wo2")
            xc0 = sb.tile([P, KC, SPB], mdt, name="xc0")

            ident = cst_sb[0:DK, 0:DK]
            ones65 = cst_sb[0:DK, DK:DK + DK + 1]

            # ---- priority-ordered DMAs (earliest-needed first) ----
            nc.gpsimd.dma_start(cst_sb[:], cst[:])
            nc.gpsimd.dma_start(wkk_sb[:], wkk3[:])
            nc.gpsimd.dma_start(wv_sb[:], wv3[:])
            nc.gpsimd.dma_start(Vp[:, :, DK], cst[:, DK:DK + NT])
            for k in range(KC):  # own 512 query rows, k-sliced for early start
                nc.gpsimd.dma_start(xc0[:, k, :], xT3[:, k, 0:SPB])
            nc.gpsimd.dma_start(wq_sb[:, :, ts(0, P)], wq3[:, :, ts(0, P)])
            nc.vector.memzero(rec64[:])

            xcs = {0: xc0}

            def aux(shape, dtype=fp32):
                return ps.tile(shape, dtype, name="aux", tag="aux", bufs=2)

            def emit_split_accum(dst_ps, stat_fn, xc_t, n_out):
                """Contract the two 64-row d_model halves on row tiles T0/T8
                into separate 1-bank psum partials (each bank owned by exactly
                one tile position; halves are merged later on the DVE)."""
                for k in range(KC):
                    first, last = (k == 0), (k == KC - 1)
                    nc.tensor.matmul(
                        dst_ps[0][0:n_out, :],
                        stat_fn(0, k), xc_t[0:DK, k, :],
                        start=first, stop=last,
                    )
                    nc.tensor.matmul(
                        dst_ps[1][0:n_out, :],
                        stat_fn(1, k), xc_t[DK:P, k, :],
                        start=first, stop=last,
                    )

            def merge_halves(dst, ps_lo, ps_hi):
                nc.vector.tensor_copy(dst, ps_hi)
                nc.vector.tensor_add(dst, ps_lo, dst)

            def emit_k2(c):
                xc_t = xcs[c]
                chunks = [aux([P, 512], fp32), aux([P, 512], fp32)]
                emit_split_accum(
                    chunks,
                    lambda h, k: wkk_sb[ts(h, DK), k, :],
                    xc_t, P,
                )
                merge_halves(
                    K2T[:, ts(c, 512)], chunks[0][:, :], chunks[1][:, :]
                )

            def emit_v(c):
                xc_t = xcs[c]
                chunks = [aux([DK + 1, 512], fp32), aux([DK + 1, 512], fp32)]
                emit_split_accum(
                    chunks,
                    lambda h, k: wv_sb[ts(h, DK), k, :],
                    xc_t, DK + 1,
                )
                vsb = sb.tile([DK, SPB], mdt, name="vsb", tag="vsb", bufs=2)
                merge_halves(vsb[:], chunks[0][0:DK, :], chunks[1][0:DK, :])
                # PE-transpose V.T -> V' [t, dv]; batched to bound mode switches
                trs = [aux([P, DK], mdt), aux([P, DK], mdt)]
                for q in range(2):
                    nc.tensor.transpose(trs[q][:], vsb[:, ts(q, P)], ident)
                for q in range(2):
                    nc.vector.tensor_copy(Vp[:, 4 * c + q, 0:DK], trs[q][:])
                trs = [aux([P, DK], mdt), aux([P, DK], mdt)]
                for q in range(2):
                    nc.tensor.transpose(trs[q][:], vsb[:, ts(q + 2, P)], ident)
                for q in range(2):
                    nc.vector.tensor_copy(Vp[:, 4 * c + q + 2, 0:DK], trs[q][:])

            def emit_q(m):
                chunks = [aux([P, 512], fp32), aux([P, 512], fp32)]
                emit_split_accum(
                    chunks,
                    lambda h, k: wq_sb[ts(h, DK), k, ts(m, P)],
                    xc0, P,
                )
                merge_halves(QT[:, m, :], chunks[0][:, :], chunks[1][:, :])

            # ---- pre-pass: K2/V for c0 and Q for m0 ----
            emit_k2(0)
            emit_v(0)
            emit_q(0)

            # ---- remaining DMAs, priority order ----
            nc.gpsimd.dma_start(wq_sb[:, :, ts(1, P)], wq3[:, :, ts(1, P)])
            xc1 = sb.tile([P, KC, SPB], mdt, name="xc", tag="xc", bufs=2)
            nc.gpsimd.dma_start(xc1[:], xT3[:, :, ts(1, SPB)])
            xcs[1] = xc1
            nc.gpsimd.dma_start(wq_sb[:, :, ts(2, P)], wq3[:, :, ts(2, P)])
            nc.gpsimd.dma_start(wo2_sb[:, 0, :], wo2[:, 0, :])
            xc2 = sb.tile([P, KC, SPB], mdt, name="xc", tag="xc", bufs=2)
            nc.gpsimd.dma_start(xc2[:], xT3[:, :, ts(2, SPB)])
            xcs[2] = xc2
            nc.gpsimd.dma_start(wq_sb[:, :, ts(3, P)], wq3[:, :, ts(3, P)])
            xc3 = sb.tile([P, KC, SPB], mdt, name="xc", tag="xc", bufs=2)
            nc.gpsimd.dma_start(xc3[:], xT3[:, :, ts(3, SPB)])
            xcs[3] = xc3
            for m in range(4, KC):
                nc.gpsimd.dma_start(wq_sb[:, :, ts(m, P)], wq3[:, :, ts(m, P)])
            for hp in range(1, H2):
                nc.gpsimd.dma_start(wo2_sb[:, hp, :], wo2[:, hp, :])

            # ---- attention passes, one head pair each ----
            def emit_av(acc, tb, ex):
                first, last = (tb == 0), (tb == NT - 1)
                for j in range(2):
                    nc.tensor.matmul(
                        acc[:, j, :], Vp[:, tb, :], ex[:, j, :],
                        start=first, stop=last,
                    )

            def emit_normalize(prev_hp, prev_acc):
                # reciprocal_approx_fast, called via _custom_dve to permit the
                # f32r (bit-identical fp32) output tile
                from concourse.dve_ops import (
                    RECIP_APPROX_FAST_CONSTS,
                    RECIPROCAL_APPROX_FAST,
                )

                # the custom-DVE reciprocal ignores AP partition offsets on
                # HW, so stage the denominator row at partition 0 first
                nc.vector.tensor_copy(den_sb[:], prev_acc[DK:DK + 1, :, :])
                c = RECIP_APPROX_FAST_CONSTS
                nc.vector._custom_dve(
                    RECIPROCAL_APPROX_FAST,
                    out=rec64[0:1, :, :],
                    in0=den_sb[:],
                    s0=c["s0"], s1=c["s1"], imm2=c["imm2"],
                )
                oT = sb.tile([P, SPB], mdt, name="oT", tag="oT", bufs=2)
                for j in range(2):
                    bc = aux([DK + 1, SPB], fp32)
                    nc.tensor.matmul(
                        bc[:], ones65, rec64[:, j, :],
                        start=True, stop=True,
                    )
                    bcs = sb.tile([DK, SPB], fp32, name="bcs", tag="bcs",
                                  bufs=2)
                    nc.vector.tensor_copy(bcs[:], bc[0:DK, :])
                    nc.vector.tensor_mul(
                        oT[ts(j, DK), :], prev_acc[0:DK, j, :], bcs[:]
                    )
                    if debug:
                        nc.gpsimd.dma_start(
                            dbcs[prev_hp, j, :, :], bcs[:]
                        )
                if debug:
                    nc.gpsimd.dma_start(doT[prev_hp, :, :], oT[:])
                return oT

            def emit_y(prev_hp, oT, sb4):
                yps = [aux([P, 512], fp32), aux([P, 512], fp32)]
                for df in range(2):
                    nc.tensor.matmul(
                        yps[df][:], oT[:, ts(sb4, P)],
                        wo2_sb[:, prev_hp, ts(df, 512)],
                        start=True, stop=True,
                    )
                for df in range(2):
                    if prev_hp == 0:
                        nc.vector.tensor_copy(
                            y_sb[:, sb4, ts(df, 512)], yps[df][:]
                        )
                    else:
                        nc.vector.tensor_add(
                            y_sb[:, sb4, ts(df, 512)], yps[df][:],
                            y_sb[:, sb4, ts(df, 512)],
                        )

            # hook schedule: {pass: {tb: [closures]}}
            hooks = {
                0: {2: [lambda: emit_k2(1)], 4: [lambda: emit_v(1)],
                    6: [lambda: emit_k2(2)], 8: [lambda: emit_v(2)],
                    10: [lambda: emit_k2(3)], 12: [lambda: emit_v(3)],
                    14: [lambda: emit_q(1)]},
            }
            for p in range(1, 7):
                hooks.setdefault(p, {})[8] = [lambda m=p + 1: emit_q(m)]

            prev = None  # (hp, acc)
            for hp in range(H2):
                acc = ps.tile([DK + 1, 2, SPB], fp32, name="acc", tag="acc",
                              bufs=1)
                exs = {}
                oT_prev = None
                for tb in range(NT):
                    sc = ps.tile([P, 2, SPB], fp32, name=f"sc{tb % 2}",
                                 tag=f"sc{tb % 2}", bufs=1)
                    for j in range(2):
                        nc.tensor.matmul(
                            sc[:, j, :],
                            K2T[ts(j, DK), ts(tb, P)],
                            QT[ts(j, DK), hp, :],
                            start=True, stop=True,
                        )
                    ex = sb.tile([P, 2, SPB], mdt, name="ex", tag="ex", bufs=4)
                    nc.scalar.activation(ex[:], sc[:], Act.Exp, scale=scale)
                    exs[tb] = ex
                    if debug and hp == 7 and tb == 0:
                        stg = sb.tile([P, 2, SPB], fp32, name="dbg_sc")
                        nc.vector.tensor_copy(stg[:], sc[:])
                        nc.gpsimd.dma_start(dsc[:], stg[:])
                        nc.gpsimd.dma_start(dex[:], ex[:])
                    if tb >= 1:
                        emit_av(acc, tb - 1, exs.pop(tb - 1))
                    # previous pass's normalize + fused output projection
                    if prev is not None:
                        if tb == 1:
                            oT_prev = emit_normalize(*prev)
                        elif 2 <= tb <= 5:
                            emit_y(prev[0], oT_prev, tb - 2)
                    for fn in hooks.get(hp, {}).get(tb, []):
                        fn()
                emit_av(acc, NT - 1, exs.pop(NT - 1))
                prev = (hp, acc)

            # tail: last pass normalize + y + writeback
            if debug:
                stg2 = sb.tile([DK + 1, 2, SPB], fp32, name="dbg_acc")
                nc.vector.tensor_copy(stg2[:], prev[1][:])
                nc.gpsimd.dma_start(dacc[:], stg2[:])
            oT_last = emit_normalize(*prev)
            for sb4 in range(NSB):
                emit_y(prev[0], oT_last, sb4)
                nc.gpsimd.dma_start(y[ts(sb4, P), :], y_sb[:, sb4, :])
            if debug:
                nc.gpsimd.dma_start(dK2T[:], K2T[:])
                nc.gpsimd.dma_start(dQT[:], QT[:])
                nc.gpsimd.dma_start(dVp[:], Vp[:])

    nc.compile()
    return nc


def make_in_maps(x, w_q, w_k, w_v, w_out):
    cst = np.zeros((P, 256), dtype=np.float32)
    cst[0:DK, 0:DK] = np.eye(DK, dtype=np.float32)
    cst[:, DK:192] = 1.0
    x = np.ascontiguousarray(np.asarray(x, dtype=np.float32))
    w_q = np.asarray(w_q, dtype=np.float32)
    w_k = np.asarray(w_k, dtype=np.float32)
    w_v = np.asarray(w_v, dtype=np.float32)
    w_out = np.asarray(w_out, dtype=np.float32)

    wqT = np.ascontiguousarray(w_q.T)
    wkkT = np.ascontiguousarray(np.concatenate([w_k.T, w_k.T], axis=1))
    wvT = np.ascontiguousarray(
        np.concatenate([w_v.T, np.zeros((D, 1), np.float32)], axis=1)
    )
    # head-pair-stacked w_out.T: wo2[phi*64+dv, hp, d] = w_out.T[(2hp+phi)*64+dv, d]
    wo2 = np.ascontiguousarray(
        w_out.T.reshape(H2, 2, DK, D).transpose(1, 2, 0, 3).reshape(P, H2, D)
    )

    in_maps = []
    for c in range(NCORES):
        b, r = divmod(c, GPB)
        # roll this core's query rows to the front; t-order is irrelevant
        # (attention sums over t), so K/V are unaffected
        xb = np.roll(x[b], -r * SPB, axis=0)
        xTc = np.ascontiguousarray(xb.T)
        in_maps.append(
            {"xT": xTc, "wqT": wqT, "wkkT": wkkT, "wvT": wvT, "wo2": wo2,
             "cst": cst}
        )
    return in_maps


_BUILD_CACHE = {}


def _cached_nc(scale: float):
    key = round(float(scale), 12)
    if key not in _BUILD_CACHE:
        _BUILD_CACHE[key] = build_bass(float(scale))
    return _BUILD_CACHE[key]


def run_on_hw(in_maps, scale, trace=False):
    from concourse.bass_utils import run_bass_kernel_spmd

    nc = _cached_nc(scale)
    return run_bass_kernel_spmd(nc, in_maps, list(range(NCORES)), trace=trace)


def assemble(results):
    out = np.empty((B, S, D), dtype=np.float32)
    for c in range(NCORES):
        b, r = divmod(c, GPB)
        out[b, r * SPB:(r + 1) * SPB] = results[c]["y"]
    return out


def kernel(x, w_q, w_k, w_v, w_out, softmax_scale):
    scale = float(np.asarray(softmax_scale).reshape(-1)[0])
    in_maps = make_in_maps(x, w_q, w_k, w_v, w_out)
    res = run_on_hw(in_maps, scale, trace=False)
    return assemble(res.results)


# revision 36
# speedup vs baseline: 1.3666x; 1.0631x over previous
"""Multi-head attention (multiquery K/V) Bass kernel for 8 trn2 NeuronCores.

Sharding: 8 cores = 2 batches x 4 query-row quarters. Each core computes the
full multiquery K/V projections for its batch (cheap, dk=64) and runs
attention + output projection for its 512 query rows over all 16 heads.
Output is a pure concatenation across cores -- no collectives.

Design (v3):
- The Scalar engine's exp over [t=2048, s=512] x 16 heads (~135us at
  1 elem/cycle/lane) is the per-core floor; everything else hides under it.
- Every steady-state matmul runs in the PE's default (128,128) mode so the
  array never drains for a tiling-mode switch:
  * scores use the twice-stacked K (K2T rows 0:64 == 64:128 == K.T) against
    zero-padded per-head Q slices (qz[j=0] = [Q_even; 0], qz[j=1] =
    [0; Q_odd]), making the contraction a full 128 rows;
  * attn@V keeps t=128 contraction with a [1|V] stationary of width 65 whose
    ones column accumulates the softmax denominator into psum row 0;
  * the fused output projection contracts the head pair (128 rows).
- 8 passes of one head pair each. PSUM: sc double buffer (4 banks) + attn@V
  accumulator (2 banks) + two 1-bank aux slots = 8 banks.
- Normalize: reciprocal_approx_fast of psum row 0 (the custom-DVE op ignores
  AP partition offsets on HW, so the denominator must live at partition 0)
  into row 0 of a zeroed [65,2,512] tile; a ones[65,65]-stationary matmul
  broadcasts it across partitions; DVE multiplies write the pair-stacked oT
  (odd head to SBUF partitions 64:128). Normalize for pass P runs before
  pass P+1's first attn@V so the accumulator hand-off never stalls exp.
- Projections for x-blocks 1..3 / q-blocks 1..7 are emitted as hooks inside
  early passes, filling PE slack under the exp cadence.
- dma_start costs ~1us of GpSimd issue time each, so only the 5 transfers
  needed by the pre-pass are issued first; the rest issue behind them.
"""

import sys

import numpy as np

if "/opt/trn_rl_repo" not in sys.path:
    sys.path.insert(0, "/opt/trn_rl_repo")

B, S, D = 2, 2048, 1024
H, DK = 16, 64
H2 = H // 2  # head pairs
P = 128
NCORES, GPB = 8, 4
SPB = S // GPB  # 512 query rows per core
KC = D // P  # 8 contraction subtiles over d_model
NT = S // P  # 16 key/t blocks
NSB = SPB // P  # 4 s blocks


def build_bass(scale: float, debug: bool = False):
    import concourse.bacc as bacc
    import concourse.mybir as mybir
    import concourse.tile as tile
    from concourse.bass import ts
    from concourse.dve_ops import (
        RECIP_APPROX_FAST_CONSTS,
        RECIPROCAL_APPROX_FAST,
    )

    fp32 = mybir.dt.float32
    mdt = mybir.dt.float32r  # fp32 bits, streams 1 cycle/row on the PE
    Act = mybir.ActivationFunctionType

    nc = bacc.Bacc(None, target_bir_lowering=False)
    xT = nc.dram_tensor("xT", [D, S], mdt, kind="ExternalInput")
    cst = nc.dram_tensor("cst", [P, 256], mdt, kind="ExternalInput")
    wqT = nc.dram_tensor("wqT", [D, D], mdt, kind="ExternalInput")
    wkkT = nc.dram_tensor("wkkT", [D, P], mdt, kind="ExternalInput")
    wvT = nc.dram_tensor("wvT", [D, DK + 1], mdt, kind="ExternalInput")
    wo2 = nc.dram_tensor("wo2", [P, H2, D], mdt, kind="ExternalInput")
    y = nc.dram_tensor("y", [SPB, D], fp32, kind="ExternalOutput")
    if debug:
        dacc = nc.dram_tensor("dacc", [DK + 1, 2, SPB], fp32,
                              kind="ExternalOutput")
        doT = nc.dram_tensor("doT", [H2, P, SPB], fp32, kind="ExternalOutput")

    xT3 = xT.rearrange("(po pi) s -> pi po s", pi=P)
    wq3 = wqT.rearrange("(po pi) d -> pi po d", pi=P)
    wkk3 = wkkT.rearrange("(po pi) d -> pi po d", pi=P)
    wv3 = wvT.rearrange("(po pi) d -> pi po d", pi=P)

    with tile.TileContext(nc) as tc:
        with (
            tc.tile_pool(name="sb", bufs=1) as sb,
            tc.tile_pool(name="ps", bufs=1, space="PSUM") as ps,
        ):
            # ---- persistent SBUF ----
            cst_sb = sb.tile([P, 256], mdt, name="cst")
            K2T = sb.tile([P, S], mdt, name="K2T")
            Vp = sb.tile([P, NT, DK + 1], mdt, name="Vp")
            qz = sb.tile([P, KC, 2, SPB], mdt, name="qz")
            rec_pad = sb.tile([P, 2, SPB], mdt, name="rec_pad")
            den_sb = sb.tile([1, 2, SPB], fp32, name="den_sb")
            y_sb = sb.tile([P, NSB, D], fp32, name="y_sb")
            wkk_sb = sb.tile([P, KC, P], mdt, name="wkk")
            wv_sb = sb.tile([P, KC, DK + 1], mdt, name="wv")
            wq_sb = sb.tile([P, KC, D], mdt, name="wq")
            wo2_sb = sb.tile([P, H2, D], mdt, name="wo2")
            xc0 = sb.tile([P, KC, SPB], mdt, name="xc0")

            ident = cst_sb[0:DK, 0:DK]
            ones65 = cst_sb[:, DK:DK + DK + 1]  # [128, 65] of ones

            # ---- critical-path DMAs (pre-pass needs only these) ----
            nc.gpsimd.dma_start(cst_sb[:], cst[:])
            nc.gpsimd.dma_start(wkk_sb[:], wkk3[:])
            nc.gpsimd.dma_start(wv_sb[:], wv3[:])
            nc.gpsimd.dma_start(xc0[:], xT3[:, :, 0:SPB])
            nc.gpsimd.dma_start(wq_sb[:, :, ts(0, P)], wq3[:, :, ts(0, P)])
            nc.vector.memzero(rec_pad[:])
            nc.vector.memzero(qz[:])

            xcs = {0: xc0}

            def aux(shape, dtype=fp32):
                return ps.tile(shape, dtype, name="aux", tag="aux", bufs=2)

            def emit_k2(c):
                xc_t = xcs[c]
                k2ps = aux([P, 512])
                for k in range(KC):
                    nc.tensor.matmul(
                        k2ps[:], wkk_sb[:, k, :], xc_t[:, k, :],
                        start=(k == 0), stop=(k == KC - 1),
                    )
                nc.vector.tensor_copy(K2T[:, ts(c, 512)], k2ps[:])

            def emit_v(c):
                xc_t = xcs[c]
                vps = aux([DK + 1, 512])
                for k in range(KC):
                    nc.tensor.matmul(
                        vps[:], wv_sb[:, k, :], xc_t[:, k, :],
                        start=(k == 0), stop=(k == KC - 1),
                    )
                vsb = sb.tile([DK, SPB], mdt, name="vsb", tag="vsb", bufs=1)
                nc.vector.tensor_copy(vsb[:], vps[0:DK, :])
                # PE-transpose V.T -> V' [t, dv] into Vp cols 0:64 (col 64 is
                # the denominator ones column); batched 2-at-a-time
                for pair in range(2):
                    trs = [aux([P, DK], mdt), aux([P, DK], mdt)]
                    for q in range(2):
                        nc.tensor.transpose(
                            trs[q][:], vsb[:, ts(2 * pair + q, P)], ident
                        )
                    for q in range(2):
                        nc.vector.tensor_copy(
                            Vp[:, 4 * c + 2 * pair + q, 0:DK], trs[q][:]
                        )

            def emit_q(m):
                qps = aux([P, 512])
                for k in range(KC):
                    nc.tensor.matmul(
                        qps[:], wq_sb[:, k, ts(m, P)], xc0[:, k, :],
                        start=(k == 0), stop=(k == KC - 1),
                    )
                # qz[j=0] = [Q_even; 0], qz[j=1] = [0; Q_odd] (zero-padded at
                # build start) so scores contract a full 128 rows
                nc.vector.tensor_copy(qz[0:DK, m, 0, :], qps[0:DK, :])
                nc.vector.tensor_copy(qz[DK:P, m, 1, :], qps[DK:P, :])

            # ---- pre-pass: K2/V for c0 and Q for m0 ----
            emit_k2(0)
            emit_v(0)
            emit_q(0)

            # ---- remaining DMAs, priority order ----
            nc.gpsimd.dma_start(Vp[:, :, DK], cst[:, DK:DK + NT])  # ones col
            xc1 = sb.tile([P, KC, SPB], mdt, name="xc", tag="xc", bufs=2)
            nc.gpsimd.dma_start(xc1[:], xT3[:, :, ts(1, SPB)])
            xcs[1] = xc1
            nc.gpsimd.dma_start(wq_sb[:, :, ts(1, P)], wq3[:, :, ts(1, P)])
            xc2 = sb.tile([P, KC, SPB], mdt, name="xc", tag="xc", bufs=2)
            nc.gpsimd.dma_start(xc2[:], xT3[:, :, ts(2, SPB)])
            xcs[2] = xc2
            nc.gpsimd.dma_start(wo2_sb[:, 0, :], wo2[:, 0, :])
            xc3 = sb.tile([P, KC, SPB], mdt, name="xc", tag="xc", bufs=2)
            nc.gpsimd.dma_start(xc3[:], xT3[:, :, ts(3, SPB)])
            xcs[3] = xc3
            nc.gpsimd.dma_start(wq_sb[:, :, ts(2, P)], wq3[:, :, ts(2, P)])
            nc.gpsimd.dma_start(wo2_sb[:, 1, :], wo2[:, 1, :])
            for m in range(3, KC):
                nc.gpsimd.dma_start(wq_sb[:, :, ts(m, P)], wq3[:, :, ts(m, P)])
            for hp in range(2, H2):
                nc.gpsimd.dma_start(wo2_sb[:, hp, :], wo2[:, hp, :])

            # ---- attention passes, one head pair each ----
            def emit_av(acc, tb, ex):
                first, last = (tb == 0), (tb == NT - 1)
                for j in range(2):
                    nc.tensor.matmul(
                        acc[:, j, :], Vp[:, tb, :], ex[:, j, :],
                        start=first, stop=last,
                    )

            def emit_normalize(prev_hp, prev_acc):
                # acc row 64 is the softmax denominator (ones column of Vp);
                # stage it at partition 0 because the custom-DVE reciprocal
                # ignores AP partition offsets on HW
                nc.vector.tensor_copy(den_sb[:], prev_acc[DK:DK + 1, :, :])
                c = RECIP_APPROX_FAST_CONSTS
                nc.vector._custom_dve(
                    RECIPROCAL_APPROX_FAST,
                    out=rec_pad[0:1, :, :],
                    in0=den_sb[:],
                    s0=c["s0"], s1=c["s1"], imm2=c["imm2"],
                )
                oT = sb.tile([P, SPB], mdt, name="oT", tag="oT", bufs=2)
                for j in range(2):
                    bc = aux([DK + 1, SPB])
                    nc.tensor.matmul(
                        bc[:], ones65, rec_pad[:, j, :], start=True, stop=True
                    )
                    bcs = sb.tile([DK, SPB], fp32, name="bcs", tag="bcs",
                                  bufs=1)
                    nc.vector.tensor_copy(bcs[:], bc[0:DK, :])
                    nc.vector.tensor_mul(
                        oT[ts(j, DK), :], prev_acc[0:DK, j, :], bcs[:]
                    )
                if debug:
                    stg = sb.tile([DK + 1, 2, SPB], fp32, name="dbg_acc")
                    nc.vector.tensor_copy(stg[:], prev_acc[:])
                    nc.gpsimd.dma_start(dacc[:], stg[:])
                    nc.gpsimd.dma_start(doT[prev_hp, :, :], oT[:])
                return oT

            def emit_y(prev_hp, oT, sb4):
                yps = [aux([P, 512]), aux([P, 512])]
                for df in range(2):
                    nc.tensor.matmul(
                        yps[df][:], oT[:, ts(sb4, P)],
                        wo2_sb[:, prev_hp, ts(df, 512)],
                        start=True, stop=True,
                    )
                for df in range(2):
                    if prev_hp == 0:
                        nc.vector.tensor_copy(
                            y_sb[:, sb4, ts(df, 512)], yps[df][:]
                        )
                    else:
                        nc.vector.tensor_add(
                            y_sb[:, sb4, ts(df, 512)], yps[df][:],
                            y_sb[:, sb4, ts(df, 512)],
                        )

            # hook schedule: {pass: {tb: [closures]}}
            hooks = {
                0: {2: [lambda: emit_k2(1)], 4: [lambda: emit_v(1)],
                    5: [lambda: emit_k2(2)], 7: [lambda: emit_v(2)],
                    9: [lambda: emit_k2(3)], 11: [lambda: emit_v(3)],
                    14: [lambda: emit_q(1)]},
            }
            for p in range(1, 7):
                hooks.setdefault(p, {})[8] = [lambda m=p + 1: emit_q(m)]

            prev = None  # (hp, acc)
            for hp in range(H2):
                acc = ps.tile([DK + 1, 2, SPB], fp32, name="acc", tag="acc",
                              bufs=1)
                exs = {}
                oT_prev = None
                for tb in range(NT):
                    sc = ps.tile([P, 2, SPB], fp32, name=f"sc{tb % 2}",
                                 tag=f"sc{tb % 2}", bufs=1)
                    for j in range(2):
                        nc.tensor.matmul(
                            sc[:, j, :],
                            K2T[:, ts(tb, P)],
                            qz[:, hp, j, :],
                            start=True, stop=True,
                        )
                    ex = sb.tile([P, 2, SPB], mdt, name="ex", tag="ex", bufs=3)
                    nc.scalar.activation(ex[:], sc[:], Act.Exp, scale=scale)
                    exs[tb] = ex
                    # normalize previous pass BEFORE this pass's first attn@V
                    # so the accumulator hand-off never blocks the exp cadence
                    if prev is not None and tb == 1:
                        oT_prev = emit_normalize(*prev)
                    if tb >= 1:
                        emit_av(acc, tb - 1, exs.pop(tb - 1))
                    if prev is not None and 2 <= tb <= 5:
                        emit_y(prev[0], oT_prev, tb - 2)
                    for fn in hooks.get(hp, {}).get(tb, []):
                        fn()
                emit_av(acc, NT - 1, exs.pop(NT - 1))
                prev = (hp, acc)

            # tail: last pass normalize + y + writeback
            oT_last = emit_normalize(*prev)
            for sb4 in range(NSB):
                emit_y(prev[0], oT_last, sb4)
                nc.gpsimd.dma_start(y[ts(sb4, P), :], y_sb[:, sb4, :])

    nc.compile()
    return nc


def make_in_maps(x, w_q, w_k, w_v, w_out):
    cst = np.zeros((P, 256), dtype=np.float32)
    cst[0:DK, 0:DK] = np.eye(DK, dtype=np.float32)
    cst[:, DK:192] = 1.0
    x = np.ascontiguousarray(np.asarray(x, dtype=np.float32))
    w_q = np.asarray(w_q, dtype=np.float32)
    w_k = np.asarray(w_k, dtype=np.float32)
    w_v = np.asarray(w_v, dtype=np.float32)
    w_out = np.asarray(w_out, dtype=np.float32)

    wqT = np.ascontiguousarray(w_q.T)
    wkkT = np.ascontiguousarray(np.concatenate([w_k.T, w_k.T], axis=1))
    wvT = np.ascontiguousarray(
        np.concatenate([w_v.T, np.zeros((D, 1), np.float32)], axis=1)
    )
    # head-pair-stacked w_out.T: wo2[phi*64+dv, hp, d] = w_out.T[(2hp+phi)*64+dv, d]
    wo2 = np.ascontiguousarray(
        w_out.T.reshape(H2, 2, DK, D).transpose(1, 2, 0, 3).reshape(P, H2, D)
    )

    in_maps = []
    for c in range(NCORES):
        b, r = divmod(c, GPB)
        # roll this core's query rows to the front; t-order is irrelevant
        # (attention sums over t), so K/V are unaffected
        xb = np.roll(x[b], -r * SPB, axis=0)
        xTc = np.ascontiguousarray(xb.T)
        in_maps.append(
            {"xT": xTc, "wqT": wqT, "wkkT": wkkT, "wvT": wvT, "wo2": wo2,
             "cst": cst}
        )
    return in_maps


_BUILD_CACHE = {}


def _cached_nc(scale: float):
    key = round(float(scale), 12)
    if key not in _BUILD_CACHE:
        _BUILD_CACHE[key] = build_bass(float(scale))
    return _BUILD_CACHE[key]


def run_on_hw(in_maps, scale, trace=False):
    from concourse.bass_utils import run_bass_kernel_spmd

    nc = _cached_nc(scale)
    return run_bass_kernel_spmd(nc, in_maps, list(range(NCORES)), trace=trace)


def assemble(results):
    out = np.empty((B, S, D), dtype=np.float32)
    for c in range(NCORES):
        b, r = divmod(c, GPB)
        out[b, r * SPB:(r + 1) * SPB] = results[c]["y"]
    return out


def kernel(x, w_q, w_k, w_v, w_out, softmax_scale):
    scale = float(np.asarray(softmax_scale).reshape(-1)[0])
    in_maps = make_in_maps(x, w_q, w_k, w_v, w_out)
    res = run_on_hw(in_maps, scale, trace=False)
    return assemble(res.results)


# revision 50
# speedup vs baseline: 1.4361x; 1.0509x over previous
"""Multi-head attention (multiquery K/V) Bass kernel for 8 trn2 NeuronCores.

Sharding: 8 cores = 2 batches x 4 query-row quarters. Each core computes the
full multiquery K/V projections for its batch (cheap, dk=64) and runs
attention + output projection for its 512 query rows over all 16 heads.
Output is a pure concatenation across cores -- no collectives.

Design (v3):
- The Scalar engine's exp over [t=2048, s=512] x 16 heads (~135us at
  1 elem/cycle/lane) is the per-core floor; everything else hides under it.
- Every steady-state matmul runs in the PE's default (128,128) mode so the
  array never drains for a tiling-mode switch:
  * scores use the twice-stacked K (K2T rows 0:64 == 64:128 == K.T) against
    zero-padded per-head Q slices (qz[j=0] = [Q_even; 0], qz[j=1] =
    [0; Q_odd]), making the contraction a full 128 rows;
  * attn@V keeps t=128 contraction with a [1|V] stationary of width 65 whose
    ones column accumulates the softmax denominator into psum row 0;
  * the fused output projection contracts the head pair (128 rows).
- 8 passes of one head pair each. PSUM: sc double buffer (4 banks) + attn@V
  accumulator (2 banks) + two 1-bank aux slots = 8 banks.
- Normalize: reciprocal_approx_fast of psum row 0 (the custom-DVE op ignores
  AP partition offsets on HW, so the denominator must live at partition 0)
  into row 0 of a zeroed [65,2,512] tile; a ones[65,65]-stationary matmul
  broadcasts it across partitions; DVE multiplies write the pair-stacked oT
  (odd head to SBUF partitions 64:128). Normalize for pass P runs before
  pass P+1's first attn@V so the accumulator hand-off never stalls exp.
- Projections for x-blocks 1..3 / q-blocks 1..7 are emitted as hooks inside
  early passes, filling PE slack under the exp cadence.
- dma_start costs ~1us of GpSimd issue time each, so only the 5 transfers
  needed by the pre-pass are issued first; the rest issue behind them.
"""

import sys

import numpy as np

if "/opt/trn_rl_repo" not in sys.path:
    sys.path.insert(0, "/opt/trn_rl_repo")

B, S, D = 2, 2048, 1024
H, DK = 16, 64
H2 = H // 2  # head pairs
P = 128
NCORES, GPB = 8, 4
SPB = S // GPB  # 512 query rows per core
KC = D // P  # 8 contraction subtiles over d_model
NT = S // P  # 16 key/t blocks
NSB = SPB // P  # 4 s blocks


def build_bass(scale: float, debug: bool = False):
    import concourse.bacc as bacc
    import concourse.mybir as mybir
    import concourse.tile as tile
    from concourse.bass import ts
    from concourse.dve_ops import (
        RECIP_APPROX_FAST_CONSTS,
        RECIPROCAL_APPROX_FAST,
    )

    fp32 = mybir.dt.float32
    mdt = mybir.dt.float32r  # fp32 bits, streams 1 cycle/row on the PE
    Act = mybir.ActivationFunctionType

    nc = bacc.Bacc(None, target_bir_lowering=False)
    xT = nc.dram_tensor("xT", [D, S], mdt, kind="ExternalInput")
    cst = nc.dram_tensor("cst", [P, 256], mdt, kind="ExternalInput")
    wqT = nc.dram_tensor("wqT", [D, D], mdt, kind="ExternalInput")
    wkkT = nc.dram_tensor("wkkT", [D, P], mdt, kind="ExternalInput")
    wvT = nc.dram_tensor("wvT", [D, DK + 1], mdt, kind="ExternalInput")
    wo2 = nc.dram_tensor("wo2", [P, H2, D], mdt, kind="ExternalInput")
    y = nc.dram_tensor("y", [SPB, D], fp32, kind="ExternalOutput")
    if debug:
        dacc = nc.dram_tensor("dacc", [DK + 1, 2, SPB], fp32,
                              kind="ExternalOutput")
        doT = nc.dram_tensor("doT", [H2, P, SPB], fp32, kind="ExternalOutput")

    xT3 = xT.rearrange("(po pi) s -> pi po s", pi=P)
    wq3 = wqT.rearrange("(po pi) d -> pi po d", pi=P)
    wkk3 = wkkT.rearrange("(po pi) d -> pi po d", pi=P)
    wv3 = wvT.rearrange("(po pi) d -> pi po d", pi=P)

    with tile.TileContext(nc) as tc:
        with (
            tc.tile_pool(name="sb", bufs=1) as sb,
            tc.tile_pool(name="ps", bufs=1, space="PSUM") as ps,
        ):
            # ---- persistent SBUF ----
            cst_sb = sb.tile([P, 256], mdt, name="cst")
            K2T = sb.tile([P, S], mdt, name="K2T")
            Vp = sb.tile([P, NT, DK + 1], mdt, name="Vp")
            qz = sb.tile([P, KC, 2, SPB], mdt, name="qz")
            rec65 = sb.tile([DK + 1, 2, SPB], mdt, name="rec65")
            den_sb = sb.tile([1, 2, SPB], fp32, name="den_sb")
            y_sb = sb.tile([P, NSB, D], fp32, name="y_sb")
            wkk_sb = sb.tile([P, KC, P], mdt, name="wkk")
            wv_sb = sb.tile([P, KC, DK + 1], mdt, name="wv")
            wq_sb = sb.tile([P, KC, D], mdt, name="wq")
            wo2_sb = sb.tile([P, H2, D], mdt, name="wo2")
            xc0 = sb.tile([P, KC, SPB], mdt, name="xc0")

            ident = cst_sb[0:DK, 0:DK]
            ones65 = cst_sb[0:DK + 1, DK:DK + DK + 1]  # [65, 65] of ones

            # ---- critical-path DMAs (pre-pass needs only these; xc0 is the
            # long pole so it issues first) ----
            nc.gpsimd.dma_start(xc0[:], xT3[:, :, 0:SPB])
            nc.gpsimd.dma_start(cst_sb[:], cst[:])
            nc.gpsimd.dma_start(wkk_sb[:], wkk3[:])
            nc.gpsimd.dma_start(wv_sb[:], wv3[:])
            nc.gpsimd.dma_start(wq_sb[:, :, ts(0, P)], wq3[:, :, ts(0, P)])
            nc.vector.memzero(rec65[:])
            nc.vector.memzero(qz[:])

            xcs = {0: xc0}

            def aux(shape, dtype=fp32):
                return ps.tile(shape, dtype, name="aux", tag="aux", bufs=2)

            # projection emitters, split into <=1us pieces so they slot into
            # per-tb PE slack without stalling the exp cadence; piece "a"
            # starts the psum accumulation, "b" finishes it and copies out
            pstate = {}

            def k2_a(c, half=None):
                k2ps = aux([P, 512])
                pstate[("k2", c)] = k2ps
                for k in range(4):
                    nc.tensor.matmul(
                        k2ps[:], wkk_sb[:, k, :], xcs[c][:, k, :],
                        start=(k == 0), stop=False,
                    )

            def k2_b(c):
                k2ps = pstate.pop(("k2", c))
                for k in range(4, KC):
                    nc.tensor.matmul(
                        k2ps[:], wkk_sb[:, k, :], xcs[c][:, k, :],
                        start=False, stop=(k == KC - 1),
                    )
                nc.vector.tensor_copy(K2T[:, ts(c, 512)], k2ps[:])

            def v_a(c):
                vps = aux([DK + 1, 512])
                pstate[("v", c)] = vps
                for k in range(4):
                    nc.tensor.matmul(
                        vps[:], wv_sb[:, k, :], xcs[c][:, k, :],
                        start=(k == 0), stop=False,
                    )

            def v_b(c):
                vps = pstate.pop(("v", c))
                for k in range(4, KC):
                    nc.tensor.matmul(
                        vps[:], wv_sb[:, k, :], xcs[c][:, k, :],
                        start=False, stop=(k == KC - 1),
                    )
                vsb = sb.tile([DK, SPB], mdt, name="vsb", tag="vsb", bufs=1)
                pstate[("vsb", c)] = vsb
                nc.vector.tensor_copy(vsb[:], vps[0:DK, :])

            def v_tr(c):
                # PE-transpose V.T -> V' [t, dv] into Vp cols 0:64 (col 64
                # is the denominator ones column)
                vsb = pstate.pop(("vsb", c))
                for pair in range(2):
                    trs = [aux([P, DK], mdt), aux([P, DK], mdt)]
                    for q in range(2):
                        nc.tensor.transpose(
                            trs[q][:], vsb[:, ts(2 * pair + q, P)], ident
                        )
                    for q in range(2):
                        nc.vector.tensor_copy(
                            Vp[:, 4 * c + 2 * pair + q, 0:DK], trs[q][:]
                        )

            def q_a(m):
                qps = aux([P, 512])
                pstate[("q", m)] = qps
                for k in range(4):
                    nc.tensor.matmul(
                        qps[:], wq_sb[:, k, ts(m, P)], xc0[:, k, :],
                        start=(k == 0), stop=False,
                    )

            def q_b(m):
                qps = pstate.pop(("q", m))
                for k in range(4, KC):
                    nc.tensor.matmul(
                        qps[:], wq_sb[:, k, ts(m, P)], xc0[:, k, :],
                        start=False, stop=(k == KC - 1),
                    )
                # qz[j=0] = [Q_even; 0], qz[j=1] = [0; Q_odd] (zero-padded at
                # build start) so scores contract a full 128 rows
                nc.vector.tensor_copy(qz[0:DK, m, 0, :], qps[0:DK, :])
                nc.vector.tensor_copy(qz[DK:P, m, 1, :], qps[DK:P, :])

            # ---- remaining DMAs (xc1 feeds the pre-pass; the rest stream
            # in behind it, ordered by first use) ----
            xc1 = sb.tile([P, KC, SPB], mdt, name="xc", tag="xc", bufs=2)
            nc.gpsimd.dma_start(xc1[:], xT3[:, :, ts(1, SPB)])
            xcs[1] = xc1
            nc.gpsimd.dma_start(Vp[:, :, DK], cst[:, DK:DK + NT])  # ones col
            xc2 = sb.tile([P, KC, SPB], mdt, name="xc", tag="xc", bufs=2)
            nc.gpsimd.dma_start(xc2[:], xT3[:, :, ts(2, SPB)])
            xcs[2] = xc2
            xc3 = sb.tile([P, KC, SPB], mdt, name="xc", tag="xc", bufs=2)
            nc.gpsimd.dma_start(xc3[:], xT3[:, :, ts(3, SPB)])
            xcs[3] = xc3
            nc.gpsimd.dma_start(wq_sb[:, :, ts(1, P)], wq3[:, :, ts(1, P)])
            nc.gpsimd.dma_start(wo2_sb[:, 0, :], wo2[:, 0, :])
            nc.gpsimd.dma_start(wq_sb[:, :, ts(2, P)], wq3[:, :, ts(2, P)])
            nc.gpsimd.dma_start(wo2_sb[:, 1, :], wo2[:, 1, :])
            for m in range(3, KC):
                nc.gpsimd.dma_start(wq_sb[:, :, ts(m, P)], wq3[:, :, ts(m, P)])
            for hp in range(2, H2):
                nc.gpsimd.dma_start(wo2_sb[:, hp, :], wo2[:, hp, :])

            # ---- pre-pass: K2/V for c0+c1 and Q for m0 ----
            k2_a(0)
            k2_b(0)
            q_a(0)
            q_b(0)
            v_a(0)
            v_b(0)
            v_tr(0)
            k2_a(1)
            k2_b(1)
            v_a(1)
            v_b(1)
            v_tr(1)

            # ---- attention passes, one head pair each ----
            def emit_av(acc, tb, ex):
                first, last = (tb == 0), (tb == NT - 1)
                for j in range(2):
                    nc.tensor.matmul(
                        acc[:, j, :], Vp[:, tb, :], ex[:, j, :],
                        start=first, stop=last,
                    )

            def norm_a(prev_hp, prev_acc):
                # acc row 64 is the softmax denominator (ones column of Vp);
                # stage it at partition 0 because the custom-DVE reciprocal
                # ignores AP partition offsets on HW.  DVE-only piece.
                nc.vector.tensor_copy(den_sb[:], prev_acc[DK:DK + 1, :, :])
                c = RECIP_APPROX_FAST_CONSTS
                nc.vector._custom_dve(
                    RECIPROCAL_APPROX_FAST,
                    out=rec65[0:1, :, :],
                    in0=den_sb[:],
                    s0=c["s0"], s1=c["s1"], imm2=c["imm2"],
                )

            def norm_b(prev_hp, prev_acc):
                # broadcast 1/den across partitions (ones-stationary matmul)
                # and write the pair-stacked normalized output oT
                oT = sb.tile([P, SPB], mdt, name="oT", tag="oT", bufs=2)
                for j in range(2):
                    bc = aux([DK + 1, SPB])
                    nc.tensor.matmul(
                        bc[:], ones65, rec65[:, j, :], start=True, stop=True
                    )
                    bcs = sb.tile([DK, SPB], fp32, name="bcs", tag="bcs",
                                  bufs=1)
                    nc.vector.tensor_copy(bcs[:], bc[0:DK, :])
                    nc.vector.tensor_mul(
                        oT[ts(j, DK), :], prev_acc[0:DK, j, :], bcs[:]
                    )
                if debug:
                    stg = sb.tile([DK + 1, 2, SPB], fp32, name="dbg_acc")
                    nc.vector.tensor_copy(stg[:], prev_acc[:])
                    nc.gpsimd.dma_start(dacc[:], stg[:])
                    nc.gpsimd.dma_start(doT[prev_hp, :, :], oT[:])
                return oT

            def emit_y(prev_hp, oT, sb4):
                yps = [aux([P, 512]), aux([P, 512])]
                for df in range(2):
                    nc.tensor.matmul(
                        yps[df][:], oT[:, ts(sb4, P)],
                        wo2_sb[:, prev_hp, ts(df, 512)],
                        start=True, stop=True,
                    )
                for df in range(2):
                    if prev_hp == 0:
                        nc.vector.tensor_copy(
                            y_sb[:, sb4, ts(df, 512)], yps[df][:]
                        )
                    else:
                        nc.vector.tensor_add(
                            y_sb[:, sb4, ts(df, 512)], yps[df][:],
                            y_sb[:, sb4, ts(df, 512)],
                        )

            # hook schedule: {pass: {tb: [closures]}} -- one <=1us piece per
            # tb so the PE never bursts past the exp cadence.  Deadlines:
            # K2T block c by sc(4c) emission; Vp block c by av(4c) (tb 4c+4);
            # qz m by the next pass's sc(0).
            hooks = {
                0: {1: [lambda: k2_a(2)], 2: [lambda: k2_b(2)],
                    3: [lambda: v_a(2)], 4: [lambda: v_b(2)],
                    5: [lambda: v_tr(2)],
                    6: [lambda: k2_a(3)], 7: [lambda: k2_b(3)],
                    8: [lambda: v_a(3)], 9: [lambda: v_b(3)],
                    10: [lambda: v_tr(3)],
                    11: [lambda: q_a(1)], 12: [lambda: q_b(1)]},
            }
            for p in range(1, 7):
                hooks.setdefault(p, {})[9] = [lambda m=p + 1: q_a(m)]
                hooks.setdefault(p, {})[10] = [lambda m=p + 1: q_b(m)]

            prev = None  # (hp, acc, tail exs 12..15)
            for hp in range(H2):
                acc = ps.tile([DK + 1, 2, SPB], fp32, name="acc", tag="acc",
                              bufs=1)
                exs = {}
                oT_prev = None
                for tb in range(NT):
                    sc = ps.tile([P, 2, SPB], fp32, name=f"sc{tb % 2}",
                                 tag=f"sc{tb % 2}", bufs=1)
                    for j in range(2):
                        nc.tensor.matmul(
                            sc[:, j, :], K2T[:, ts(tb, P)], qz[:, hp, j, :],
                            start=True, stop=True,
                        )
                    # attn@V runs 3 tb behind exp (ex bufs=3); emitted before
                    # the activation so the freed ex slot is ready in time
                    if tb == 0 and prev is not None:
                        emit_av(prev[1], 13, prev[2].pop(13))
                    if tb >= 3:
                        emit_av(acc, tb - 3, exs.pop(tb - 3))
                    ex = sb.tile([P, 2, SPB], mdt, name="ex", tag="ex", bufs=3)
                    nc.scalar.activation(ex[:], sc[:], Act.Exp, scale=scale)
                    exs[tb] = ex
                    if tb == 0 and prev is not None:
                        emit_av(prev[1], 14, prev[2].pop(14))
                        emit_av(prev[1], 15, prev[2].pop(15))
                    if prev is not None:
                        if tb == 1:
                            norm_a(*prev[:2])
                        elif tb == 2:
                            oT_prev = norm_b(*prev[:2])
                        elif 5 <= tb <= 8:
                            emit_y(prev[0], oT_prev, tb - 5)
                    for fn in hooks.get(hp, {}).get(tb, []):
                        fn()
                prev = (hp, acc, exs)

            # tail: last pass's deferred attn@V + normalize + y + writeback
            for tb in range(13, NT):
                emit_av(prev[1], tb, prev[2].pop(tb))
            norm_a(*prev[:2])
            oT_last = norm_b(*prev[:2])
            for sb4 in range(NSB):
                emit_y(prev[0], oT_last, sb4)
                nc.sync.dma_start(y[ts(sb4, P), :], y_sb[:, sb4, :])

    nc.compile()
    return nc


def make_in_maps(x, w_q, w_k, w_v, w_out):
    cst = np.zeros((P, 256), dtype=np.float32)
    cst[0:DK, 0:DK] = np.eye(DK, dtype=np.float32)
    cst[:, DK:192] = 1.0
    x = np.ascontiguousarray(np.asarray(x, dtype=np.float32))
    w_q = np.asarray(w_q, dtype=np.float32)
    w_k = np.asarray(w_k, dtype=np.float32)
    w_v = np.asarray(w_v, dtype=np.float32)
    w_out = np.asarray(w_out, dtype=np.float32)

    wqT = np.ascontiguousarray(w_q.T)
    wkkT = np.ascontiguousarray(np.concatenate([w_k.T, w_k.T], axis=1))
    wvT = np.ascontiguousarray(
        np.concatenate([w_v.T, np.zeros((D, 1), np.float32)], axis=1)
    )
    # head-pair-stacked w_out.T: wo2[phi*64+dv, hp, d] = w_out.T[(2hp+phi)*64+dv, d]
    wo2 = np.ascontiguousarray(
        w_out.T.reshape(H2, 2, DK, D).transpose(1, 2, 0, 3).reshape(P, H2, D)
    )

    in_maps = []
    for c in range(NCORES):
        b, r = divmod(c, GPB)
        # roll this core's query rows to the front; t-order is irrelevant
        # (attention sums over t), so K/V are unaffected
        xb = np.roll(x[b], -r * SPB, axis=0)
        xTc = np.ascontiguousarray(xb.T)
        in_maps.append(
            {"xT": xTc, "wqT": wqT, "wkkT": wkkT, "wvT": wvT, "wo2": wo2,
             "cst": cst}
        )
    return in_maps


_BUILD_CACHE = {}


def _cached_nc(scale: float):
    key = round(float(scale), 12)
    if key not in _BUILD_CACHE:
        _BUILD_CACHE[key] = build_bass(float(scale))
    return _BUILD_CACHE[key]


def run_on_hw(in_maps, scale, trace=False):
    from concourse.bass_utils import run_bass_kernel_spmd

    nc = _cached_nc(scale)
    return run_bass_kernel_spmd(nc, in_maps, list(range(NCORES)), trace=trace)


def assemble(results):
    out = np.empty((B, S, D), dtype=np.float32)
    for c in range(NCORES):
        b, r = divmod(c, GPB)
        out[b, r * SPB:(r + 1) * SPB] = results[c]["y"]
    return out


def kernel(x, w_q, w_k, w_v, w_out, softmax_scale):
    scale = float(np.asarray(softmax_scale).reshape(-1)[0])
    in_maps = make_in_maps(x, w_q, w_k, w_v, w_out)
    res = run_on_hw(in_maps, scale, trace=False)
    return assemble(res.results)


# revision 60
# speedup vs baseline: 1.6658x; 1.1600x over previous
"""Multi-head attention (multiquery K/V) Bass kernel for 8 trn2 NeuronCores.

Sharding: 8 cores = 2 batches x 4 query-row quarters. Each core computes the
full multiquery K/V projections for its batch (cheap, dk=64) and runs
attention + output projection for its 512 query rows over all 16 heads.
Output is a pure concatenation across cores -- no collectives.

Design (v3):
- The Scalar engine's exp over [t=2048, s=512] x 16 heads (~135us at
  1 elem/cycle/lane) is the per-core floor; everything else hides under it.
- Every steady-state matmul runs in the PE's default (128,128) mode so the
  array never drains for a tiling-mode switch:
  * scores use the twice-stacked K (K2T rows 0:64 == 64:128 == K.T) against
    zero-padded per-head Q slices (qz[j=0] = [Q_even; 0], qz[j=1] =
    [0; Q_odd]), making the contraction a full 128 rows;
  * attn@V keeps t=128 contraction with a [1|V] stationary of width 65 whose
    ones column accumulates the softmax denominator into psum row 0;
  * the fused output projection contracts the head pair (128 rows).
- 8 passes of one head pair each. PSUM: sc double buffer (4 banks) + attn@V
  accumulator (2 banks) + two 1-bank aux slots = 8 banks.
- Normalize: reciprocal_approx_fast of psum row 0 (the custom-DVE op ignores
  AP partition offsets on HW, so the denominator must live at partition 0)
  into row 0 of a zeroed [65,2,512] tile; a ones[65,65]-stationary matmul
  broadcasts it across partitions; DVE multiplies write the pair-stacked oT
  (odd head to SBUF partitions 64:128). Normalize for pass P runs before
  pass P+1's first attn@V so the accumulator hand-off never stalls exp.
- Projections for x-blocks 1..3 / q-blocks 1..7 are emitted as hooks inside
  early passes, filling PE slack under the exp cadence.
- dma_start costs ~1us of GpSimd issue time each, so only the 5 transfers
  needed by the pre-pass are issued first; the rest issue behind them.
"""

import sys

import numpy as np

if "/opt/trn_rl_repo" not in sys.path:
    sys.path.insert(0, "/opt/trn_rl_repo")

B, S, D = 2, 2048, 1024
H, DK = 16, 64
H2 = H // 2  # head pairs
P = 128
NCORES, GPB = 8, 4
SPB = S // GPB  # 512 query rows per core
KC = D // P  # 8 contraction subtiles over d_model
NT = S // P  # 16 key/t blocks
NSB = SPB // P  # 4 s blocks


def build_bass(scale: float, debug: bool = False):
    import concourse.bacc as bacc
    import concourse.mybir as mybir
    import concourse.tile as tile
    from concourse.bass import ts
    from concourse.dve_ops import (
        RECIP_APPROX_FAST_CONSTS,
        RECIPROCAL_APPROX_FAST,
    )

    fp32 = mybir.dt.float32
    mdt = mybir.dt.float32r  # fp32 bits, streams 1 cycle/row on the PE
    Act = mybir.ActivationFunctionType

    nc = bacc.Bacc(None, target_bir_lowering=False)
    xT = nc.dram_tensor("xT", [D, S], mdt, kind="ExternalInput")
    cst = nc.dram_tensor("cst", [P, 256], mdt, kind="ExternalInput")
    wqT = nc.dram_tensor("wqT", [D, D], mdt, kind="ExternalInput")
    wkkT = nc.dram_tensor("wkkT", [D, P], mdt, kind="ExternalInput")
    wvT = nc.dram_tensor("wvT", [D, DK + 1], mdt, kind="ExternalInput")
    wo2 = nc.dram_tensor("wo2", [P, H2, D], mdt, kind="ExternalInput")
    y = nc.dram_tensor("y", [SPB, D], fp32, kind="ExternalOutput")
    if debug:
        dacc = nc.dram_tensor("dacc", [DK + 1, 2, SPB], fp32,
                              kind="ExternalOutput")
        doT = nc.dram_tensor("doT", [H2, P, SPB], fp32, kind="ExternalOutput")

    xT3 = xT.rearrange("(po pi) s -> pi po s", pi=P)
    wq3 = wqT.rearrange("(po pi) d -> pi po d", pi=P)
    wkk3 = wkkT.rearrange("(po pi) d -> pi po d", pi=P)
    wv3 = wvT.rearrange("(po pi) d -> pi po d", pi=P)

    with tile.TileContext(nc) as tc:
        with (
            tc.tile_pool(name="sb", bufs=1) as sb,
            tc.tile_pool(name="ps", bufs=1, space="PSUM") as ps,
        ):
            # ---- persistent SBUF ----
            bf16 = mybir.dt.bfloat16
            cst_sb = sb.tile([P, 256], mdt, name="cst")
            K2T = sb.tile([P, S], bf16, name="K2T")
            # Vp stationary is 96 wide: col 0 = denominator ones column,
            # cols 32:96 = V', so attn@V lands den at psum row 0 (readable by
            # the offset-dropping custom reciprocal) and V at the aligned
            # rows 32:96
            Vp = sb.tile([P, NT, 96], mdt, name="Vp")
            qz = sb.tile([P, KC, 2, SPB], bf16, name="qz")
            rec65 = sb.tile([DK + 1, 2, SPB], mdt, name="rec65")
            acc_sb = sb.tile([96, 2, SPB], fp32, name="acc_sb")
            y_sb = sb.tile([P, NSB, D], fp32, name="y_sb")
            wkk_sb = sb.tile([P, KC, P], mdt, name="wkk")
            wv_sb = sb.tile([P, KC, DK + 1], mdt, name="wv")
            wq_sb = sb.tile([P, KC, D], mdt, name="wq")
            wo2_sb = sb.tile([P, H2, D], mdt, name="wo2")
            xc0 = sb.tile([P, KC, SPB], mdt, name="xc0")

            ident = cst_sb[0:DK, 0:DK]
            ones96 = cst_sb[0:DK + 1, DK:DK + 96]  # [65, 96] of ones

            # ---- critical-path DMAs (pre-pass needs only these; xc0 is the
            # long pole so it issues first) ----
            nc.gpsimd.dma_start(xc0[:], xT3[:, :, 0:SPB])
            nc.gpsimd.dma_start(cst_sb[:], cst[:])
            nc.gpsimd.dma_start(wkk_sb[:], wkk3[:])
            nc.gpsimd.dma_start(wv_sb[:], wv3[:])
            nc.gpsimd.dma_start(wq_sb[:, :, ts(0, P)], wq3[:, :, ts(0, P)])
            nc.vector.memzero(rec65[:])
            nc.vector.memzero(qz[:])
            nc.vector.memzero(Vp[:])

            xcs = {0: xc0}

            def aux(shape, dtype=fp32):
                return ps.tile(shape, dtype, name="aux", tag="aux", bufs=2)

            # projection emitters, split into <=1us pieces so they slot into
            # per-tb PE slack without stalling the exp cadence; piece "a"
            # starts the psum accumulation, "b" finishes it and copies out
            pstate = {}

            def k2_a(c, half=None):
                k2ps = aux([P, 512])
                pstate[("k2", c)] = k2ps
                for k in range(4):
                    nc.tensor.matmul(
                        k2ps[:], wkk_sb[:, k, :], xcs[c][:, k, :],
                        start=(k == 0), stop=False,
                    )

            def k2_b(c):
                k2ps = pstate.pop(("k2", c))
                for k in range(4, KC):
                    nc.tensor.matmul(
                        k2ps[:], wkk_sb[:, k, :], xcs[c][:, k, :],
                        start=False, stop=(k == KC - 1),
                    )
                nc.vector.tensor_copy(K2T[:, ts(c, 512)], k2ps[:])

            def v_a(c):
                vps = aux([DK + 1, 512])
                pstate[("v", c)] = vps
                for k in range(4):
                    nc.tensor.matmul(
                        vps[:], wv_sb[:, k, :], xcs[c][:, k, :],
                        start=(k == 0), stop=False,
                    )

            def v_b(c):
                vps = pstate.pop(("v", c))
                for k in range(4, KC):
                    nc.tensor.matmul(
                        vps[:], wv_sb[:, k, :], xcs[c][:, k, :],
                        start=False, stop=(k == KC - 1),
                    )
                vsb = sb.tile([DK, SPB], mdt, name="vsb", tag="vsb", bufs=1)
                pstate[("vsb", c)] = vsb
                nc.vector.tensor_copy(vsb[:], vps[0:DK, :])

            def v_tr(c):
                # PE-transpose V.T -> V' [t, dv] into Vp cols 0:64 (col 64
                # is the denominator ones column)
                vsb = pstate.pop(("vsb", c))
                for pair in range(2):
                    trs = [aux([P, DK], mdt), aux([P, DK], mdt)]
                    for q in range(2):
                        nc.tensor.transpose(
                            trs[q][:], vsb[:, ts(2 * pair + q, P)], ident
                        )
                    for q in range(2):
                        nc.vector.tensor_copy(
                            Vp[:, 4 * c + 2 * pair + q, 32:96], trs[q][:]
                        )

            def q_a(m):
                qps = aux([P, 512])
                pstate[("q", m)] = qps
                for k in range(4):
                    nc.tensor.matmul(
                        qps[:], wq_sb[:, k, ts(m, P)], xc0[:, k, :],
                        start=(k == 0), stop=False,
                    )

            def q_b(m):
                qps = pstate.pop(("q", m))
                for k in range(4, KC):
                    nc.tensor.matmul(
                        qps[:], wq_sb[:, k, ts(m, P)], xc0[:, k, :],
                        start=False, stop=(k == KC - 1),
                    )
                # qz[j=0] = [Q_even; 0], qz[j=1] = [0; Q_odd] (zero-padded at
                # build start) so scores contract a full 128 rows
                nc.vector.tensor_copy(qz[0:DK, m, 0, :], qps[0:DK, :])
                nc.vector.tensor_copy(qz[DK:P, m, 1, :], qps[DK:P, :])

            # ---- remaining DMAs (xc1 feeds the pre-pass; the rest stream
            # in behind it, ordered by first use) ----
            xc1 = sb.tile([P, KC, SPB], mdt, name="xc", tag="xc", bufs=2)
            nc.gpsimd.dma_start(xc1[:], xT3[:, :, ts(1, SPB)])
            xcs[1] = xc1
            nc.gpsimd.dma_start(Vp[:, :, 0], cst[:, DK:DK + NT])  # ones col
            xc2 = sb.tile([P, KC, SPB], mdt, name="xc", tag="xc", bufs=2)
            nc.gpsimd.dma_start(xc2[:], xT3[:, :, ts(2, SPB)])
            xcs[2] = xc2
            xc3 = sb.tile([P, KC, SPB], mdt, name="xc", tag="xc", bufs=2)
            nc.gpsimd.dma_start(xc3[:], xT3[:, :, ts(3, SPB)])
            xcs[3] = xc3
            nc.gpsimd.dma_start(wq_sb[:, :, ts(1, P)], wq3[:, :, ts(1, P)])
            nc.gpsimd.dma_start(wo2_sb[:, 0, :], wo2[:, 0, :])
            nc.gpsimd.dma_start(wq_sb[:, :, ts(2, P)], wq3[:, :, ts(2, P)])
            nc.gpsimd.dma_start(wo2_sb[:, 1, :], wo2[:, 1, :])
            for m in range(3, KC):
                nc.gpsimd.dma_start(wq_sb[:, :, ts(m, P)], wq3[:, :, ts(m, P)])
            for hp in range(2, H2):
                nc.gpsimd.dma_start(wo2_sb[:, hp, :], wo2[:, hp, :])

            # ---- pre-pass: K2/V for c0+c1 and Q for m0 ----
            k2_a(0)
            k2_b(0)
            q_a(0)
            q_b(0)
            v_a(0)
            v_b(0)
            v_tr(0)
            k2_a(1)
            k2_b(1)
            v_a(1)
            v_b(1)
            v_tr(1)

            # ---- attention passes, one head pair each ----
            def emit_av(acc, tb, ex):
                first, last = (tb == 0), (tb == NT - 1)
                for j in range(2):
                    nc.tensor.matmul(
                        acc[:, j, :], Vp[:, tb, :], ex[:, j, :],
                        start=first, stop=last,
                    )

            def norm_a(prev_hp):
                # reciprocal of the denominator row (psum row 0 -> acc_sb
                # row 0, the tile base, so the offset-dropping custom op
                # reads the right partition)
                c = RECIP_APPROX_FAST_CONSTS
                nc.vector._custom_dve(
                    RECIPROCAL_APPROX_FAST,
                    out=rec65[0:1, :, :],
                    in0=acc_sb[0:1, :, :],
                    s0=c["s0"], s1=c["s1"], imm2=c["imm2"],
                )

            def norm_b(prev_hp):
                # broadcast 1/den across partitions (ones-stationary matmul)
                # and write the pair-stacked normalized output oT
                oT = sb.tile([P, SPB], mdt, name="oT", tag="oT", bufs=2)
                for j in range(2):
                    bc = aux([96, SPB])
                    nc.tensor.matmul(
                        bc[:], ones96, rec65[:, j, :], start=True, stop=True
                    )
                    bcs = sb.tile([96, SPB], fp32, name="bcs", tag="vsb",
                                  bufs=1)
                    nc.vector.tensor_copy(bcs[:], bc[:])
                    # 32-row pieces: spans starting at partition 32/96 may
                    # cover at most 32 partitions, and SBUF+SBUF inputs must
                    # share a base partition
                    for hf in range(2):
                        nc.vector.tensor_mul(
                            oT[j * DK + hf * 32:j * DK + hf * 32 + 32, :],
                            acc_sb[32 + hf * 32:64 + hf * 32, j, :],
                            bcs[32 + hf * 32:64 + hf * 32, :],
                        )
                if debug:
                    nc.gpsimd.dma_start(dacc[:], acc_sb[0:DK + 1, :, :])
                    nc.gpsimd.dma_start(doT[prev_hp, :, :], oT[:])
                return oT

            def emit_y(prev_hp, oT, sb4):
                yps = [aux([P, 512]), aux([P, 512])]
                for df in range(2):
                    nc.tensor.matmul(
                        yps[df][:], oT[:, ts(sb4, P)],
                        wo2_sb[:, prev_hp, ts(df, 512)],
                        start=True, stop=True,
                    )
                for df in range(2):
                    if prev_hp == 0:
                        nc.vector.tensor_copy(
                            y_sb[:, sb4, ts(df, 512)], yps[df][:]
                        )
                    else:
                        nc.vector.tensor_add(
                            y_sb[:, sb4, ts(df, 512)], yps[df][:],
                            y_sb[:, sb4, ts(df, 512)],
                        )

            # hook schedule: {pass: {tb: [closures]}} -- one <=1us piece per
            # tb so the PE never bursts past the exp cadence.  Deadlines:
            # K2T block c by sc(4c) emission; Vp block c by av(4c) (tb 4c+4);
            # qz m by the next pass's sc(0).
            hooks = {
                0: {1: [lambda: k2_a(2)], 2: [lambda: k2_b(2)],
                    3: [lambda: v_a(2)], 4: [lambda: v_b(2)],
                    5: [lambda: v_tr(2)],
                    6: [lambda: k2_a(3)], 7: [lambda: k2_b(3)],
                    8: [lambda: v_a(3)], 9: [lambda: v_b(3)],
                    10: [lambda: v_tr(3)],
                    11: [lambda: q_a(1)], 12: [lambda: q_b(1)]},
            }
            for p in range(1, 7):
                hooks.setdefault(p, {})[9] = [lambda m=p + 1: q_a(m)]
                hooks.setdefault(p, {})[10] = [lambda m=p + 1: q_b(m)]

            prev = None  # (hp, acc, tail exs 13..15)
            for hp in range(H2):
                acc = ps.tile([96, 2, SPB], fp32, name="acc", tag="acc",
                              bufs=1)
                exs = {}
                oT_prev = None
                for tb in range(NT):
                    sc = ps.tile([P, 2, SPB], fp32, name=f"sc{tb % 2}",
                                 tag=f"sc{tb % 2}", bufs=1)
                    for j in range(2):
                        nc.tensor.matmul(
                            sc[:, j, :], K2T[:, ts(tb, P)], qz[:, hp, j, :],
                            start=True, stop=True,
                        )
                    # attn@V runs 3 tb behind exp (ex bufs=3); emitted before
                    # the activation so the freed ex slot is ready in time
                    if tb == 0 and prev is not None:
                        emit_av(prev[1], 13, prev[2].pop(13))
                    if tb >= 3:
                        emit_av(acc, tb - 3, exs.pop(tb - 3))
                    ex = sb.tile([P, 2, SPB], mdt, name="ex", tag="ex", bufs=3)
                    nc.scalar.activation(ex[:], sc[:], Act.Exp, scale=scale)
                    exs[tb] = ex
                    if tb == 0 and prev is not None:
                        emit_av(prev[1], 14, prev[2].pop(14))
                        emit_av(prev[1], 15, prev[2].pop(15))
                        # evacuate the finished accumulator so this pass's
                        # attn@V can claim the psum banks 3 tb from now; the
                        # normalize then works from the SBUF copy at leisure
                        nc.vector.tensor_copy(acc_sb[:], prev[1][:])
                    if prev is not None:
                        if tb == 1:
                            norm_a(prev[0])
                        elif tb == 2:
                            oT_prev = norm_b(prev[0])
                        elif 5 <= tb <= 8:
                            emit_y(prev[0], oT_prev, tb - 5)
                    for fn in hooks.get(hp, {}).get(tb, []):
                        fn()
                prev = (hp, acc, exs)

            # tail: last pass's deferred attn@V + normalize + y + writeback
            for tb in range(13, NT):
                emit_av(prev[1], tb, prev[2].pop(tb))
            nc.vector.tensor_copy(acc_sb[:], prev[1][:])
            norm_a(prev[0])
            oT_last = norm_b(prev[0])
            for sb4 in range(NSB):
                emit_y(prev[0], oT_last, sb4)
                nc.sync.dma_start(y[ts(sb4, P), :], y_sb[:, sb4, :])

    nc.compile()
    return nc


def make_in_maps(x, w_q, w_k, w_v, w_out):
    cst = np.zeros((P, 256), dtype=np.float32)
    cst[0:DK, 0:DK] = np.eye(DK, dtype=np.float32)
    cst[:, DK:192] = 1.0
    x = np.ascontiguousarray(np.asarray(x, dtype=np.float32))
    w_q = np.asarray(w_q, dtype=np.float32)
    w_k = np.asarray(w_k, dtype=np.float32)
    w_v = np.asarray(w_v, dtype=np.float32)
    w_out = np.asarray(w_out, dtype=np.float32)

    wqT = np.ascontiguousarray(w_q.T)
    wkkT = np.ascontiguousarray(np.concatenate([w_k.T, w_k.T], axis=1))
    wvT = np.ascontiguousarray(
        np.concatenate([w_v.T, np.zeros((D, 1), np.float32)], axis=1)
    )
    # head-pair-stacked w_out.T: wo2[phi*64+dv, hp, d] = w_out.T[(2hp+phi)*64+dv, d]
    wo2 = np.ascontiguousarray(
        w_out.T.reshape(H2, 2, DK, D).transpose(1, 2, 0, 3).reshape(P, H2, D)
    )

    in_maps = []
    for c in range(NCORES):
        b, r = divmod(c, GPB)
        # roll this core's query rows to the front; t-order is irrelevant
        # (attention sums over t), so K/V are unaffected
        xb = np.roll(x[b], -r * SPB, axis=0)
        xTc = np.ascontiguousarray(xb.T)
        in_maps.append(
            {"xT": xTc, "wqT": wqT, "wkkT": wkkT, "wvT": wvT, "wo2": wo2,
             "cst": cst}
        )
    return in_maps


_BUILD_CACHE = {}


def _cached_nc(scale: float):
    key = round(float(scale), 12)
    if key not in _BUILD_CACHE:
        _BUILD_CACHE[key] = build_bass(float(scale))
    return _BUILD_CACHE[key]


def run_on_hw(in_maps, scale, trace=False):
    from concourse.bass_utils import run_bass_kernel_spmd

    nc = _cached_nc(scale)
    return run_bass_kernel_spmd(nc, in_maps, list(range(NCORES)), trace=trace)


def assemble(results):
    out = np.empty((B, S, D), dtype=np.float32)
    for c in range(NCORES):
        b, r = divmod(c, GPB)
        out[b, r * SPB:(r + 1) * SPB] = results[c]["y"]
    return out


def kernel(x, w_q, w_k, w_v, w_out, softmax_scale):
    scale = float(np.asarray(softmax_scale).reshape(-1)[0])
    in_maps = make_in_maps(x, w_q, w_k, w_v, w_out)
    res = run_on_hw(in_maps, scale, trace=False)
    return assemble(res.results)


# revision 72
# speedup vs baseline: 1.7347x; 1.0414x over previous
"""Multi-head attention (multiquery K/V) Bass kernel for 8 trn2 NeuronCores.

Sharding: 8 cores = 2 batches x 4 query-row quarters. Each core computes the
full multiquery K/V projections for its batch (cheap, dk=64) and runs
attention + output projection for its 512 query rows over all 16 heads.
Output is a pure concatenation across cores -- no collectives.

Design (v3):
- The Scalar engine's exp over [t=2048, s=512] x 16 heads (~135us at
  1 elem/cycle/lane) is the per-core floor; everything else hides under it.
- Every steady-state matmul runs in the PE's default (128,128) mode so the
  array never drains for a tiling-mode switch:
  * scores use the twice-stacked K (K2T rows 0:64 == 64:128 == K.T) against
    zero-padded per-head Q slices (qz[j=0] = [Q_even; 0], qz[j=1] =
    [0; Q_odd]), making the contraction a full 128 rows;
  * attn@V keeps t=128 contraction with a [1|V] stationary of width 65 whose
    ones column accumulates the softmax denominator into psum row 0;
  * the fused output projection contracts the head pair (128 rows).
- 8 passes of one head pair each. PSUM: sc double buffer (4 banks) + attn@V
  accumulator (2 banks) + two 1-bank aux slots = 8 banks.
- Normalize: reciprocal_approx_fast of psum row 0 (the custom-DVE op ignores
  AP partition offsets on HW, so the denominator must live at partition 0)
  into row 0 of a zeroed [65,2,512] tile; a ones[65,65]-stationary matmul
  broadcasts it across partitions; DVE multiplies write the pair-stacked oT
  (odd head to SBUF partitions 64:128). Normalize for pass P runs before
  pass P+1's first attn@V so the accumulator hand-off never stalls exp.
- Projections for x-blocks 1..3 / q-blocks 1..7 are emitted as hooks inside
  early passes, filling PE slack under the exp cadence.
- dma_start costs ~1us of GpSimd issue time each, so only the 5 transfers
  needed by the pre-pass are issued first; the rest issue behind them.
"""

import sys

import numpy as np

if "/opt/trn_rl_repo" not in sys.path:
    sys.path.insert(0, "/opt/trn_rl_repo")

B, S, D = 2, 2048, 1024
H, DK = 16, 64
H2 = H // 2  # head pairs
P = 128
NCORES, GPB = 8, 4
SPB = S // GPB  # 512 query rows per core
KC = D // P  # 8 contraction subtiles over d_model
NT = S // P  # 16 key/t blocks
NSB = SPB // P  # 4 s blocks


def build_bass(scale: float, debug: bool = False):
    import concourse.bacc as bacc
    import concourse.mybir as mybir
    import concourse.tile as tile
    from concourse.bass import ts
    from concourse.dve_ops import (
        RECIP_APPROX_FAST_CONSTS,
        RECIPROCAL_APPROX_FAST,
    )

    fp32 = mybir.dt.float32
    mdt = mybir.dt.float32r  # fp32 bits, streams 1 cycle/row on the PE
    Act = mybir.ActivationFunctionType

    bf16 = mybir.dt.bfloat16
    nc = bacc.Bacc(None, target_bir_lowering=False)
    xT = nc.dram_tensor("xT", [D, S], bf16, kind="ExternalInput")
    cst = nc.dram_tensor("cst", [P, 256], mdt, kind="ExternalInput")
    wqT = nc.dram_tensor("wqT", [D, D], bf16, kind="ExternalInput")
    wkkT = nc.dram_tensor("wkkT", [D, P], bf16, kind="ExternalInput")
    wvT = nc.dram_tensor("wvT", [D, DK + 1], bf16, kind="ExternalInput")
    wo2 = nc.dram_tensor("wo2", [P, H2, D], mdt, kind="ExternalInput")
    y = nc.dram_tensor("y", [SPB, D], fp32, kind="ExternalOutput")
    if debug:
        dacc = nc.dram_tensor("dacc", [DK + 1, 2, SPB], fp32,
                              kind="ExternalOutput")
        doT = nc.dram_tensor("doT", [H2, P, SPB], fp32, kind="ExternalOutput")

    xT3 = xT.rearrange("(po pi) s -> pi po s", pi=P)
    wq3 = wqT.rearrange("(po pi) d -> pi po d", pi=P)
    wkk3 = wkkT.rearrange("(po pi) d -> pi po d", pi=P)
    wv3 = wvT.rearrange("(po pi) d -> pi po d", pi=P)

    with tile.TileContext(nc) as tc:
        with (
            tc.tile_pool(name="sb", bufs=1) as sb,
            tc.tile_pool(name="ps", bufs=1, space="PSUM") as ps,
        ):
            # ---- persistent SBUF ----
            cst_sb = sb.tile([P, 256], mdt, name="cst")
            K2T = sb.tile([P, S], bf16, name="K2T")
            # Vp stationary is 96 wide: col 0 = denominator ones column,
            # cols 32:96 = V', so attn@V lands den at psum row 0 (readable by
            # the offset-dropping custom reciprocal) and V at the aligned
            # rows 32:96
            Vp = sb.tile([P, NT, 96], mdt, name="Vp")
            qz = sb.tile([P, KC, 2, SPB], bf16, name="qz")
            rec65 = sb.tile([DK + 1, 2, SPB], mdt, name="rec65")
            acc_sb = sb.tile([96, 2, SPB], fp32, name="acc_sb")
            y_sb = sb.tile([P, NSB, D], fp32, name="y_sb")
            wkk_sb = sb.tile([P, KC, P], bf16, name="wkk")
            wv_sb = sb.tile([P, KC, DK + 1], bf16, name="wv")
            wq_sb = sb.tile([P, KC, D], bf16, name="wq")
            wo2_sb = sb.tile([P, H2, D], mdt, name="wo2")
            xc0 = sb.tile([P, KC, SPB], bf16, name="xc0")

            ident = cst_sb[0:DK, 0:DK]
            ones96 = cst_sb[0:DK + 1, DK:DK + 96]  # [65, 96] of ones

            # ---- critical-path DMAs (pre-pass needs only these; xc0 is the
            # long pole so it issues first) ----
            nc.gpsimd.dma_start(xc0[:], xT3[:, :, 0:SPB])
            nc.gpsimd.dma_start(cst_sb[:], cst[:])
            nc.gpsimd.dma_start(wkk_sb[:], wkk3[:])
            nc.gpsimd.dma_start(wv_sb[:], wv3[:])
            nc.gpsimd.dma_start(wq_sb[:, :, ts(0, P)], wq3[:, :, ts(0, P)])
            nc.vector.memzero(rec65[:])
            nc.vector.memzero(qz[:])
            nc.vector.memzero(Vp[:])

            xcs = {0: xc0}

            def aux(shape, dtype=fp32):
                return ps.tile(shape, dtype, name="aux", tag="aux", bufs=2)

            # projection emitters, split into <=1us pieces so they slot into
            # per-tb PE slack without stalling the exp cadence; piece "a"
            # starts the psum accumulation, "b" finishes it and copies out
            pstate = {}

            def k2_a(c, half=None):
                k2ps = aux([P, 512])
                pstate[("k2", c)] = k2ps
                for k in range(4):
                    nc.tensor.matmul(
                        k2ps[:], wkk_sb[:, k, :], xcs[c][:, k, :],
                        start=(k == 0), stop=False,
                    )

            def k2_b(c):
                k2ps = pstate.pop(("k2", c))
                for k in range(4, KC):
                    nc.tensor.matmul(
                        k2ps[:], wkk_sb[:, k, :], xcs[c][:, k, :],
                        start=False, stop=(k == KC - 1),
                    )
                nc.vector.tensor_copy(K2T[:, ts(c, 512)], k2ps[:])

            def v_a(c):
                vps = aux([DK + 1, 512])
                pstate[("v", c)] = vps
                for k in range(4):
                    nc.tensor.matmul(
                        vps[:], wv_sb[:, k, :], xcs[c][:, k, :],
                        start=(k == 0), stop=False,
                    )

            def v_b(c):
                vps = pstate.pop(("v", c))
                for k in range(4, KC):
                    nc.tensor.matmul(
                        vps[:], wv_sb[:, k, :], xcs[c][:, k, :],
                        start=False, stop=(k == KC - 1),
                    )
                vsb = sb.tile([DK, SPB], mdt, name="vsb", tag="vsb", bufs=1)
                pstate[("vsb", c)] = vsb
                nc.vector.tensor_copy(vsb[:], vps[0:DK, :])

            def v_tr(c):
                # PE-transpose V.T -> V' [t, dv] into Vp cols 0:64 (col 64
                # is the denominator ones column)
                vsb = pstate.pop(("vsb", c))
                for pair in range(2):
                    trs = [aux([P, DK], mdt), aux([P, DK], mdt)]
                    for q in range(2):
                        nc.tensor.transpose(
                            trs[q][:], vsb[:, ts(2 * pair + q, P)], ident
                        )
                    for q in range(2):
                        nc.vector.tensor_copy(
                            Vp[:, 4 * c + 2 * pair + q, 32:96], trs[q][:]
                        )

            def q_a(m):
                qps = aux([P, 512])
                pstate[("q", m)] = qps
                for k in range(4):
                    nc.tensor.matmul(
                        qps[:], wq_sb[:, k, ts(m, P)], xc0[:, k, :],
                        start=(k == 0), stop=False,
                    )

            def q_b(m):
                qps = pstate.pop(("q", m))
                for k in range(4, KC):
                    nc.tensor.matmul(
                        qps[:], wq_sb[:, k, ts(m, P)], xc0[:, k, :],
                        start=False, stop=(k == KC - 1),
                    )
                # qz[j=0] = [Q_even; 0], qz[j=1] = [0; Q_odd] (zero-padded at
                # build start) so scores contract a full 128 rows
                nc.vector.tensor_copy(qz[0:DK, m, 0, :], qps[0:DK, :])
                nc.vector.tensor_copy(qz[DK:P, m, 1, :], qps[DK:P, :])

            # ---- remaining DMAs (xc1 feeds the pre-pass; the rest stream
            # in behind it, ordered by first use) ----
            xc1 = sb.tile([P, KC, SPB], bf16, name="xc", tag="xc", bufs=2)
            nc.gpsimd.dma_start(xc1[:], xT3[:, :, ts(1, SPB)])
            xcs[1] = xc1
            nc.gpsimd.dma_start(wq_sb[:, :, ts(1, P)], wq3[:, :, ts(1, P)])
            nc.gpsimd.dma_start(Vp[:, :, 0], cst[:, DK:DK + NT])  # ones col
            xc2 = sb.tile([P, KC, SPB], bf16, name="xc", tag="xc", bufs=2)
            nc.gpsimd.dma_start(xc2[:], xT3[:, :, ts(2, SPB)])
            xcs[2] = xc2
            xc3 = sb.tile([P, KC, SPB], bf16, name="xc", tag="xc", bufs=2)
            nc.gpsimd.dma_start(xc3[:], xT3[:, :, ts(3, SPB)])
            xcs[3] = xc3
            nc.gpsimd.dma_start(wo2_sb[:, 0, :], wo2[:, 0, :])
            nc.gpsimd.dma_start(wq_sb[:, :, ts(2, P)], wq3[:, :, ts(2, P)])
            nc.gpsimd.dma_start(wo2_sb[:, 1, :], wo2[:, 1, :])
            for m in range(3, KC):
                nc.gpsimd.dma_start(wq_sb[:, :, ts(m, P)], wq3[:, :, ts(m, P)])
            for hp in range(2, H2):
                nc.gpsimd.dma_start(wo2_sb[:, hp, :], wo2[:, hp, :])

            # ---- pre-pass: K2/V for c0 and Q for m0, m1 ----
            k2_a(0)
            k2_b(0)
            q_a(0)
            q_b(0)
            q_a(1)
            q_b(1)
            v_a(0)
            v_b(0)
            v_tr(0)

            # ---- attention passes, one head pair each ----
            def emit_av(acc, tb, ex):
                first, last = (tb == 0), (tb == NT - 1)
                for j in range(2):
                    nc.tensor.matmul(
                        acc[:, j, :], Vp[:, tb, :], ex[:, j, :],
                        start=first, stop=last,
                    )

            def norm_a(prev_hp):
                # reciprocal of the denominator row (psum row 0 -> acc_sb
                # row 0, the tile base, so the offset-dropping custom op
                # reads the right partition)
                c = RECIP_APPROX_FAST_CONSTS
                nc.vector._custom_dve(
                    RECIPROCAL_APPROX_FAST,
                    out=rec65[0:1, :, :],
                    in0=acc_sb[0:1, :, :],
                    s0=c["s0"], s1=c["s1"], imm2=c["imm2"],
                )

            def norm_b(prev_hp):
                # broadcast 1/den across partitions (ones-stationary matmul)
                # and write the pair-stacked normalized output oT
                oT = sb.tile([P, SPB], mdt, name="oT", tag="oT", bufs=2)
                for j in range(2):
                    bc = aux([96, SPB])
                    nc.tensor.matmul(
                        bc[:], ones96, rec65[:, j, :], start=True, stop=True
                    )
                    bcs = sb.tile([96, SPB], fp32, name="bcs", tag="vsb",
                                  bufs=1)
                    nc.vector.tensor_copy(bcs[:], bc[:])
                    # 32-row pieces: spans starting at partition 32/96 may
                    # cover at most 32 partitions, and SBUF+SBUF inputs must
                    # share a base partition
                    for hf in range(2):
                        nc.vector.tensor_mul(
                            oT[j * DK + hf * 32:j * DK + hf * 32 + 32, :],
                            acc_sb[32 + hf * 32:64 + hf * 32, j, :],
                            bcs[32 + hf * 32:64 + hf * 32, :],
                        )
                if debug:
                    nc.gpsimd.dma_start(dacc[:], acc_sb[0:DK + 1, :, :])
                    nc.gpsimd.dma_start(doT[prev_hp, :, :], oT[:])
                return oT

            def emit_y(prev_hp, oT, sb4):
                yps = [aux([P, 512]), aux([P, 512])]
                for df in range(2):
                    nc.tensor.matmul(
                        yps[df][:], oT[:, ts(sb4, P)],
                        wo2_sb[:, prev_hp, ts(df, 512)],
                        start=True, stop=True,
                    )
                for df in range(2):
                    if prev_hp == 0:
                        nc.vector.tensor_copy(
                            y_sb[:, sb4, ts(df, 512)], yps[df][:]
                        )
                    else:
                        nc.vector.tensor_add(
                            y_sb[:, sb4, ts(df, 512)], yps[df][:],
                            y_sb[:, sb4, ts(df, 512)],
                        )

            def q_p(m, i):
                if i == 0:
                    pstate[("q", m)] = aux([P, 512])
                qps = pstate[("q", m)]
                for k in (2 * i, 2 * i + 1):
                    nc.tensor.matmul(
                        qps[:], wq_sb[:, k, ts(m, P)], xc0[:, k, :],
                        start=(k == 0), stop=(k == KC - 1),
                    )

            def q_fin(m):
                qps = pstate.pop(("q", m))
                nc.vector.tensor_copy(qz[0:DK, m, 0, :], qps[0:DK, :])
                nc.vector.tensor_copy(qz[DK:P, m, 1, :], qps[DK:P, :])

            # hook schedule: {pass: {tb: [closures]}} -- one <=1us piece per
            # tb so the PE never bursts past the exp cadence.  Deadlines:
            # K2T block c by sc(4c) emission; Vp block c by av(4c) (tb 4c+4,
            # attn@V lag 4); qz m by the next pass's sc(0).
            hooks = {
                0: {1: [lambda: k2_a(1)], 2: [lambda: k2_b(1)],
                    3: [lambda: v_a(1)], 4: [lambda: v_b(1)],
                    5: [lambda: v_tr(1)],
                    6: [lambda: k2_a(2)], 7: [lambda: k2_b(2)],
                    8: [lambda: v_a(2)], 9: [lambda: v_b(2)],
                    10: [lambda: v_tr(2), lambda: k2_a(3)],
                    11: [lambda: k2_b(3)],
                    12: [lambda: v_a(3)], 13: [lambda: v_b(3)],
                    14: [lambda: v_tr(3)]},
            }
            for p in range(1, 7):
                for i in range(4):
                    hooks.setdefault(p, {})[8 + i] = [
                        lambda m=p + 1, i=i: q_p(m, i)
                    ]
                hooks.setdefault(p, {})[12] = [lambda m=p + 1: q_fin(m)]

            prev = None  # (hp, acc, tail exs 13..15)
            for hp in range(H2):
                acc = ps.tile([96, 2, SPB], fp32, name="acc", tag="acc",
                              bufs=1)
                exs = {}
                oT_prev = None
                for tb in range(NT):
                    sc = ps.tile([P, 2, SPB], fp32, name=f"sc{tb % 2}",
                                 tag=f"sc{tb % 2}", bufs=1)
                    for j in range(2):
                        nc.tensor.matmul(
                            sc[:, j, :], K2T[:, ts(tb, P)], qz[:, hp, j, :],
                            start=True, stop=True,
                        )
                    # attn@V runs 4 tb behind exp (ex bufs=4); emitted before
                    # the activation so the freed ex slot is ready in time
                    if tb == 0 and prev is not None:
                        emit_av(prev[1], 12, prev[2].pop(12))
                        emit_av(prev[1], 13, prev[2].pop(13))
                    if tb >= 4:
                        emit_av(acc, tb - 4, exs.pop(tb - 4))
                    ex = sb.tile([P, 2, SPB], mdt, name="ex", tag="ex", bufs=4)
                    nc.scalar.activation(ex[:], sc[:], Act.Exp, scale=scale)
                    exs[tb] = ex
                    if tb == 0 and prev is not None:
                        emit_av(prev[1], 14, prev[2].pop(14))
                        emit_av(prev[1], 15, prev[2].pop(15))
                        # evacuate the finished accumulator so this pass's
                        # attn@V can claim the psum banks 4 tb from now; the
                        # normalize then works from the SBUF copy at leisure
                        nc.vector.tensor_copy(acc_sb[:], prev[1][:])
                    if prev is not None:
                        if tb == 1:
                            norm_a(prev[0])
                        elif tb == 2:
                            oT_prev = norm_b(prev[0])
                        elif 5 <= tb <= 8:
                            emit_y(prev[0], oT_prev, tb - 5)
                    for fn in hooks.get(hp, {}).get(tb, []):
                        fn()
                prev = (hp, acc, exs)

            # tail: last pass's deferred attn@V + normalize + y + writeback.
            # y partials rotate through the dead sc banks as well as aux so
            # the four output blocks pipeline instead of serializing.
            for tb in range(12, NT):
                emit_av(prev[1], tb, prev[2].pop(tb))
            nc.vector.tensor_copy(acc_sb[:], prev[1][:])
            norm_a(prev[0])
            oT_last = norm_b(prev[0])
            for sb4 in range(NSB):
                yps = [
                    ps.tile([P, 512], fp32, name="typs", tag=f"sc{sb4 % 2}",
                            bufs=1),
                    aux([P, 512]),
                ]
                for df in range(2):
                    nc.tensor.matmul(
                        yps[df][:], oT_last[:, ts(sb4, P)],
                        wo2_sb[:, H2 - 1, ts(df, 512)],
                        start=True, stop=True,
                    )
                for df in range(2):
                    nc.vector.tensor_add(
                        y_sb[:, sb4, ts(df, 512)], yps[df][:],
                        y_sb[:, sb4, ts(df, 512)],
                    )
                nc.sync.dma_start(y[ts(sb4, P), :], y_sb[:, sb4, :])

    nc.compile()
    return nc


def make_in_maps(x, w_q, w_k, w_v, w_out):
    import ml_dtypes

    bf16 = ml_dtypes.bfloat16
    cst = np.zeros((P, 256), dtype=np.float32)
    cst[0:DK, 0:DK] = np.eye(DK, dtype=np.float32)
    cst[:, DK:192] = 1.0
    x = np.ascontiguousarray(np.asarray(x, dtype=np.float32))
    w_q = np.asarray(w_q, dtype=np.float32)
    w_k = np.asarray(w_k, dtype=np.float32)
    w_v = np.asarray(w_v, dtype=np.float32)
    w_out = np.asarray(w_out, dtype=np.float32)

    wqT = np.ascontiguousarray(w_q.T.astype(bf16))
    wkkT = np.ascontiguousarray(
        np.concatenate([w_k.T, w_k.T], axis=1).astype(bf16)
    )
    wvT = np.ascontiguousarray(
        np.concatenate([w_v.T, np.zeros((D, 1), np.float32)], axis=1)
        .astype(bf16)
    )
    # head-pair-stacked w_out.T: wo2[phi*64+dv, hp, d] = w_out.T[(2hp+phi)*64+dv, d]
    wo2 = np.ascontiguousarray(
        w_out.T.reshape(H2, 2, DK, D).transpose(1, 2, 0, 3).reshape(P, H2, D)
    )

    in_maps = []
    for c in range(NCORES):
        b, r = divmod(c, GPB)
        # roll this core's query rows to the front; t-order is irrelevant
        # (attention sums over t), so K/V are unaffected
        xb = np.roll(x[b], -r * SPB, axis=0)
        xTc = np.ascontiguousarray(xb.T.astype(bf16))
        in_maps.append(
            {"xT": xTc, "wqT": wqT, "wkkT": wkkT, "wvT": wvT, "wo2": wo2,
             "cst": cst}
        )
    return in_maps


_BUILD_CACHE = {}


def _cached_nc(scale: float):
    key = round(float(scale), 12)
    if key not in _BUILD_CACHE:
        _BUILD_CACHE[key] = build_bass(float(scale))
    return _BUILD_CACHE[key]


def run_on_hw(in_maps, scale, trace=False):
    from concourse.bass_utils import run_bass_kernel_spmd

    nc = _cached_nc(scale)
    return run_bass_kernel_spmd(nc, in_maps, list(range(NCORES)), trace=trace)


def assemble(results):
    out = np.empty((B, S, D), dtype=np.float32)
    for c in range(NCORES):
        b, r = divmod(c, GPB)
        out[b, r * SPB:(r + 1) * SPB] = results[c]["y"]
    return out


def kernel(x, w_q, w_k, w_v, w_out, softmax_scale):
    scale = float(np.asarray(softmax_scale).reshape(-1)[0])
    in_maps = make_in_maps(x, w_q, w_k, w_v, w_out)
    res = run_on_hw(in_maps, scale, trace=False)
    return assemble(res.results)
